# revision 1
# baseline (speedup 1.0000x reference)
"""Linear-chain CRF forward pass on 8 Trainium2 NeuronCores.

Reference recurrence (per batch element b):
    alpha_t[j] = x_t[j] + logsumexp_k(alpha_{t-1}[k] + trans[j,k])
    out[b] = sum_j alpha_{L_b - 1}[j]

Exp-space device formulation with a constant per-step log shift c folded
into the transition matrix:
    E_t = (Mc @ E_{t-1}) * X_t,  Mc[j,k] = exp(trans[j,k] - c),  X_t = exp(x_t)
so alpha_t = log E_r + r*c + A for a per-trajectory constant A.

The 2048-step serial chain is cut into SEG=64 segments of 32 steps, each
warmed up W=2 rounds from an arbitrary positive init (the Birkhoff
contraction of the positive map E -> Mc @ E converges the direction; the
unknown per-segment log offsets A_s are recovered on the host by
telescoping class-mean log-ratios at segment boundaries, which also
cancels most of the residual warmup error).

Per-core layout (32 batch elements/core, data-parallel over batch):
  - X = exp(x) is precomputed on the HOST and shipped as bf16; this
    removes the device ACT-exp pass entirely.  Round-0 state IS the
    round-0 X tile (no init op).
  - State E[128, 1024] bf16 in a 6-deep ring: 128 partitions = 2
    row-blocks x 64 classes; a column slot holds one (segment, local_b)
    pair.  Slots are permuted per core so the 256 global extraction
    events (sorted by extraction round) sit in contiguous columns 0..255
    of row-block 0: extraction is one small ACT range-copy per round
    instead of 256 point copies.
  - 4 chains (independent column ranges), each advancing one recurrence
    step per round via a bf16 block-diagonal matmul (PE) then an
    elementwise multiply:
      chains 0,1 (435 cols each): DVE multiplies straight out of PSUM --
        DVE is the only multiplier that can read PSUM (GPSIMD cannot
        access PSUM; ACT has no tensor*tensor) and runs ~100%% busy;
      chains 2,3 (77 cols each): ACT copies PSUM->SBUF bf16, then Pool
        (gpsimd) multiplies from SBUF.  Each hop chain has its own PSUM
        bank -- PE streaming into one half of a shared bank while ACT
        reads the other half wedges real hardware (works in CoreSim).
  - Dual segment grids: batch elements whose endpoint falls late within
    a grid-A segment (t* %% 32 >= 26) use grid B (segments offset by 16)
    so every extraction lands by round ~29 and the fin DMA overlaps the
    last rounds of compute; grid-B chains bridge to the exact segment 0
    through an extra round-14 snapshot.  Snapshot DMAs (3 full-state
    dumps; the round-0 snapshot is the X init the host already holds)
    and the fin DMA issue from SP between statically-placed chunk DMAs;
    X streams in a 4-deep ring of 2-round chunks with a 1-round chunk up
    front so round 1 is gated by the smallest possible transfer.
  - PE runs small filler matmuls to hold the high p-state; matmul issue
    order (2,0,3,1) matches steady-state dependency readiness.
"""

from contextlib import ExitStack

import numpy as np

B, T, C = 256, 2048, 64
NCORES = 8
BPC = B // NCORES          # 32
SEG = 64
SL = T // SEG              # 32 steps per segment
W = 2                      # warmup rounds (grid-A segment lead-in)
L = SL + 1                 # rounds; round 0 = init; set per-input by _schedule
NCOL = 1024                # state columns (= SEG * BPC * C / 128)
DCH = 2                    # rounds per X DMA chunk
# snapshot rounds for host stitching; round 0 needs no device snapshot --
# the round-0 state IS the X init, which the host already has in xp.
# Round 14 bridges grid-B chains to the exact segment 0; rounds 30/32 pair
# consecutive segments of grids A/B respectively.
SNAP_ROUNDS = (0, 14, SL - 2, SL)           # (0, 14, 30, 32)
# chain column ranges: [0, ND), [ND, 2*ND) multiplied directly from PSUM on
# DVE; two NP "hop" ranges whose PSUM is first copied to SBUF by ACT (GPSIMD
# cannot access PSUM) and then multiplied on Pool from SBUF
ND = 435
NP = (NCOL - 2 * ND) // 2  # 77
CHAIN_COLS = ((0, ND), (ND, ND), (2 * ND, NP), (2 * ND + NP, NP))
NSX = 3
RING = 4
ED = 6
PRE_FILL = 2               # PE p-state pre-ramp fillers before round 1
# per-round PE issue plan: chain index or 'F' (filler), in order
PE_PLAN = (2, 0, 3, 1, 'F', 'F')

_CACHE = {}


def _c_step(transitions, pad_x):
    """Mean per-step growth of max_j alpha, from a short host simulation."""
    x = np.asarray(pad_x[:4], np.float64)
    tr = np.asarray(transitions, np.float64)
    a = x[:, 0, :]
    tot, n = 0.0, 0
    for t in range(1, 257):
        s = a[:, None, :] + tr[None, :, :]
        m = s.max(axis=2, keepdims=True)
        a_new = x[:, t, :] + np.log(np.exp(s - m).sum(axis=2)) + m[:, :, 0]
        tot += float((a_new.max(axis=1) - a.max(axis=1)).mean())
        n += 1
        a = a_new
    return tot / n


def _schedule(batch_sizes):
    """Global extraction events sorted by round, dual-grid.

    A batch element whose endpoint falls late within a grid-A segment
    (t* %% 32 >= 26) is moved to grid B (segments offset by 16), where it
    extracts at an early round -- this empties the last rounds of
    extraction events so the fin DMA overlaps compute.  Also sets the
    global round count L = max(SL+1, max event round + 1).

    Returns (events, slot_of, grids): events[e] = (round, core, local_b,
    seg_index); grids[core, local_b] = True for grid B."""
    global L
    bs = np.asarray(batch_sizes).reshape(NCORES, BPC)
    grids = np.zeros((NCORES, BPC), bool)
    ev = []
    for core in range(NCORES):
        for b in range(BPC):
            tstar = int(bs[core, b]) - 1
            if tstar >= 26 and tstar % 32 >= 26 and tstar <= SL * SEG - 7:
                grids[core, b] = True
                i = (tstar + 16) // 32
                r = tstar - 32 * i + 18
            elif tstar < SL:
                i, r = 0, tstar
            else:
                i = tstar // SL
                r = tstar - SL * i + W
            ev.append((r, core, b, i))
    ev.sort(key=lambda t: (t[0], t[1], t[2]))
    L = max(SL + 1, max(e[0] for e in ev) + 1)
    slot_of = {(e[1], e[2]): i for i, e in enumerate(ev)}
    return ev, slot_of, grids


def _slot_maps(events, slot_of):
    """Per-core bijection (seg, local_b) -> slot index in [0, 2*NCOL).

    Slot index s: row-block h = s // NCOL, column = s % NCOL.  Event slots
    (own (seg_b, b) pairs) are pinned at their global event index; all
    other pairs fill the remaining slots in order."""
    maps = np.empty((NCORES, SEG, BPC), np.int32)
    nev = len(events)
    for core in range(NCORES):
        pinned = {}
        for b in range(BPC):
            e = slot_of[(core, b)]
            s = events[e][3]
            pinned[(s, b)] = e
        used = set(pinned.values())
        free = iter([i for i in range(2 * NCOL) if i not in used])
        for s in range(SEG):
            for b in range(BPC):
                maps[core, s, b] = pinned.get((s, b), -1)
        it = np.nditer(maps[core], flags=['multi_index'], op_flags=['readwrite'])
        for v in it:
            if v == -1:
                v[...] = next(free)
    return maps


def _build_host_inputs(pad_x, transitions, origination, c, maps, grids):
    """xp per core: [128, L*NCOL] bf16 of exp(x) in (round, slot) layout,
    plus block-diagonal bf16 weights [128, 128]."""
    import ml_dtypes
    mc = np.exp(np.asarray(transitions, np.float64) - c).astype(np.float32)
    wmat = np.zeros((128, 128), ml_dtypes.bfloat16)
    wmat[:64, :64] = mc.T.astype(ml_dtypes.bfloat16)  # lhsT[k, j] = Mc[j, k]
    wmat[64:, 64:] = mc.T.astype(ml_dtypes.bfloat16)

    x0 = np.asarray(pad_x, np.float32).copy()
    x0[:, 0, :] += np.asarray(origination, np.float32)[None, :]
    xc = x0.reshape(NCORES, BPC, T, C)

    xraw = np.empty((NCORES, 128, L, NCOL), ml_dtypes.bfloat16)
    inv = np.empty((NCORES, 2 * NCOL, 2), np.int32)      # slot -> (seg, b)
    for core in range(NCORES):
        m = maps[core].reshape(-1)
        inv_c = np.empty((2 * NCOL, 2), np.int32)
        inv_c[m, 0] = np.repeat(np.arange(SEG), BPC)
        inv_c[m, 1] = np.tile(np.arange(BPC), SEG)
        inv[core] = inv_c
        # block per slot: exp(x[b_slot, t(s_slot, r), :]) masked to 1;
        # segment i of grid A starts at 32i-2, of grid B at 32i-18
        segs = inv_c[:, 0]
        bs_ = inv_c[:, 1]
        off = np.where(grids[core][bs_], 18, W)
        t0 = np.where(segs == 0, 0, segs * SL - off)     # (2*NCOL,)
        t_idx = t0[:, None] + np.arange(L)[None, :]      # (2*NCOL, L)
        valid = (t_idx >= 0) & (t_idx < T)
        t_clip = np.clip(t_idx, 0, T - 1)
        blk = xc[core][bs_[:, None], t_clip, :]          # (2*NCOL, L, C)
        blk = np.where(valid[:, :, None], blk, 0.0)
        eb = np.exp(blk).astype(ml_dtypes.bfloat16)      # (2*NCOL, L, C)
        xraw[core, :64] = eb[:NCOL].transpose(2, 1, 0)
        xraw[core, 64:] = eb[NCOL:].transpose(2, 1, 0)
    return xraw.reshape(NCORES, 128, L * NCOL), wmat, inv


def _ranges_by_round(events):
    """round -> (e_start, n) contiguous event-column range."""
    out = {}
    for e, (r, _, _, _) in enumerate(events):
        if r not in out:
            out[r] = [e, 0]
        out[r][1] += 1
    return {r: (a, n) for r, (a, n) in out.items()}


def _build_program(rby):
    """Raw-bass program with explicit per-engine streams.

    rby: round -> (event_col_start, n) extraction ranges (global union)."""
    import concourse.bass as bass
    from concourse import mybir

    dt = mybir.dt
    # chunk k covers rounds [CB[k], CB[k+1]); chunk 0 is a 1-round mini
    # chunk so round 1 is gated by the smallest possible transfer
    CB = [1, 2]
    while CB[-1] < L:
        CB.append(min(CB[-1] + DCH, L))
    NCHUNK = len(CB) - 1
    chunk_of = [0] * L
    for k in range(NCHUNK):
        for r in range(CB[k], CB[k + 1]):
            chunk_of[r] = k
    nc = bass.Bass()
    xp = nc.declare_dram_parameter("xp", [128, L * NCOL], dt.bfloat16, False)
    wm = nc.declare_dram_parameter("wm", [128, 128], dt.bfloat16, False)
    snaps = nc.declare_dram_parameter(
        "snaps", [len(SNAP_ROUNDS) - 1, 128, NCOL], dt.bfloat16, True)
    assert len(SNAP_ROUNDS) == 4
    fin = nc.declare_dram_parameter("fin", [64, B], dt.bfloat16, True)

    nev_ops = len(rby)
    # fin split: everything extracted by round L-5 goes in an early DMA that
    # overlaps the last rounds of compute; the remainder in a tiny final DMA
    FIN_OPS1 = sum(1 for r in rby if r <= L - 5)
    FIN_A1 = max((a + n for r, (a, n) in rby.items() if r <= L - 5),
                 default=0)
    if FIN_A1 > B - 16:
        FIN_A1 = 0          # tail remainder too small to be worth a DMA

    with ExitStack() as ctx:
        def sb(name, shape, d):
            return ctx.enter_context(nc.sbuf_tensor(name, shape, d))
        w = sb("w", [128, 128], dt.bfloat16)
        rawa = sb("rawa", [128, NCOL], dt.bfloat16)   # round-0 X (init state)
        raw = [sb(f"raw{i}", [128, DCH * NCOL], dt.bfloat16)
               for i in range(RING)]
        et = [sb(f"et{i}", [128, NCOL], dt.bfloat16) for i in range(ED)]
        hb = [sb(f"hb{i}", [128, 2 * NP], dt.bfloat16)
              for i in range(2)] if NP else None
        fin_t = sb("fin_t", [64, B], dt.bfloat16)
        psd = [ctx.enter_context(
            nc.psum_tensor(f"psd{c}", [128, ND], dt.float32))
            for c in range(2)]
        psp = [ctx.enter_context(
            nc.psum_tensor(f"psp{c}", [128, NP], dt.float32))
            for c in (2, 3)] if NP else None
        psf = ctx.enter_context(nc.psum_tensor("psf", [128, 128], dt.float32))
        s_w = ctx.enter_context(nc.semaphore("s_w"))
        s_xa = ctx.enter_context(nc.semaphore("s_xa"))
        s_x = tuple(ctx.enter_context(nc.semaphore(f"s_x{i}"))
                    for i in range(NSX))
        s_pd = ctx.enter_context(nc.semaphore("s_pd"))   # PE -> DVE chains
        s_pp = ctx.enter_context(nc.semaphore("s_pp"))   # PE -> hop copies
        s_h = ctx.enter_context(nc.semaphore("s_h"))     # ACT hop copies
        s_vd = ctx.enter_context(nc.semaphore("s_vd"))   # DVE muls
        s_vp = ctx.enter_context(nc.semaphore("s_vp"))   # Pool muls
        s_f = ctx.enter_context(nc.semaphore("s_f"))     # ACT ext copies
        s_o = ctx.enter_context(nc.semaphore("s_o"))     # snap DMA completions
        block = ctx.enter_context(nc.Block())

        # cumulative ACT ext-op count through round r (for WAR waits)
        cum_ext = [0] * L
        for r in range(L):
            cum_ext[r] = (cum_ext[r - 1] if r else 0) + (1 if r in rby else 0)

        def xsl(r, c0, n):
            k = chunk_of[r]
            rr = r - CB[k]
            return raw[k % RING][:, rr * NCOL + c0:rr * NCOL + c0 + n]

        def chunk_arrived(eng, r):
            k = chunk_of[r]
            eng.wait_ge(s_x[k % NSX], 16 * (k // NSX + 1))

        @block.sync
        def _(sync):
            sync.dma_start(rawa[:], xp[:, 0:NCOL]).then_inc(s_xa, 16)
            for k in range(NCHUNK):
                if k >= RING:
                    # raw slot k-RING reuse: all muls of its rounds retired
                    r_last = CB[k - RING + 1] - 1
                    sync.wait_ge(s_vd, 2 * r_last)
                    if NP:
                        sync.wait_ge(s_vp, 2 * r_last)
                if k >= NSX:
                    sync.wait_ge(s_x[k % NSX], 16 * (k // NSX))
                r0, r1 = CB[k], CB[k + 1]
                sync.dma_start(
                    raw[k % RING][:, :(r1 - r0) * NCOL],
                    xp[:, r0 * NCOL:r1 * NCOL],
                ).then_inc(s_x[k % NSX], 16)
                if k == 9:
                    # bridge snapshot (round 14) for grid-B chains; chunk 10
                    # is needed ~5 rounds after this wait clears, so the
                    # chunk pipeline cannot starve
                    r_s = SNAP_ROUNDS[1]
                    sync.wait_ge(s_vd, 2 * r_s)
                    if NP:
                        sync.wait_ge(s_vp, 2 * r_s)
                    sync.dma_start(snaps[0],
                                   et[r_s % ED][:]).then_inc(s_o, 16)
            r_s = SNAP_ROUNDS[2]
            sync.wait_ge(s_vd, 2 * r_s)
            if NP:
                sync.wait_ge(s_vp, 2 * r_s)
            sync.dma_start(snaps[1], et[r_s % ED][:]).then_inc(s_o, 16)
            if FIN_A1 and FIN_A1 < B:
                sync.wait_ge(s_f, FIN_OPS1)
                sync.dma_start(fin[:, 0:FIN_A1],
                               fin_t[:, 0:FIN_A1]).then_inc(s_o, 16)
            sync.wait_ge(s_f, nev_ops)
            if FIN_A1 and FIN_A1 < B:
                sync.dma_start(fin[:, FIN_A1:], fin_t[:, FIN_A1:]).then_inc(
                    s_o, 16)
            else:
                sync.dma_start(fin[:, :], fin_t[:]).then_inc(s_o, 16)
            # the round-(L-1) snapshot is unavoidably last: adjacent
            # same-offset segments can only be compared at round 32
            r_s = SNAP_ROUNDS[3]
            sync.wait_ge(s_vd, 2 * r_s)
            if NP:
                sync.wait_ge(s_vp, 2 * r_s)
            sync.dma_start(snaps[2], et[r_s % ED][:]).then_inc(s_o, 16)

        @block.scalar
        def _(scalar):
            # ACT: per-round PSUM->SBUF hop copies for the Pool chains, plus
            # the extraction range-copies, in round order.
            nc.scalar.dma_start(w[:], wm[:, :]).then_inc(s_w, 16)
            if 0 in rby:
                a, n = rby[0]
                scalar.wait_ge(s_xa, 16)
                nc.scalar.copy(fin_t[:, a:a + n],
                               rawa[0:64, a:a + n]).then_inc(s_f, 1)
            for r in range(1, L):
                for c in ((2, 3) if NP else ()):
                    pc0 = 0 if c == 2 else NP
                    # hb WAR (pool mul of r-2 done) is implied: this copy
                    # waits mm(c,r) which waits mul(c,r-1) > mul(c,r-2)
                    scalar.wait_ge(s_pp, 2 * (r - 1) + (c - 2) + 1)
                    nc.scalar.copy(
                        hb[r % 2][:, pc0:pc0 + NP],
                        psp[c - 2][:]).then_inc(s_h, 1)
                if r in rby:
                    a, n = rby[r]
                    # chain-0 mul of round r done (ext cols are in chain 0)
                    scalar.wait_ge(s_vd, 2 * (r - 1) + 1)
                    nc.scalar.copy(fin_t[:, a:a + n],
                                   et[r % ED][0:64, a:a + n]).then_inc(s_f, 1)

        @block.tensor
        def _(tensor):
            def filler(n=1):
                for _ in range(n):
                    nc.tensor.matmul(psf[:], w[:], w[:, 0:128],
                                     start=True, stop=True)

            tensor.wait_ge(s_w, 16)
            filler(PRE_FILL)      # p-state pre-ramp while round-0 X lands
            first_real = next(c for c in PE_PLAN if c != 'F')
            for r in range(1, L):
                # issue plan matches steady-state dependency readiness
                for c in PE_PLAN:
                    if c == 'F':
                        filler()
                        continue
                    c0, n = CHAIN_COLS[c]
                    if r == 1:
                        if c == first_real:
                            tensor.wait_ge(s_xa, 16)
                        mov = rawa[:, c0:c0 + n]
                    else:
                        if c < 2:
                            tensor.wait_ge(s_vd, 2 * (r - 1) - 1 + c)
                        else:
                            tensor.wait_ge(s_vp, 2 * (r - 1) - 1 + (c - 2))
                        mov = et[(r - 1) % ED][:, c0:c0 + n]
                    if c < 2:
                        ps_out = psd[c][:]
                        sem = s_pd
                    else:
                        ps_out = psp[c - 2][:]
                        sem = s_pp
                    nc.tensor.matmul(ps_out, w[:], mov,
                                     start=True, stop=True).then_inc(sem, 1)

        @block.vector
        def _(vector):
            for r in range(1, L):
                if r == CB[chunk_of[r]]:
                    chunk_arrived(vector, r)
                for c in range(2):
                    c0, n = CHAIN_COLS[c]
                    if c == 0:
                        # WAR: ext copy (ACT) of round r-ED reads the et slot
                        # chain 0 is about to overwrite
                        if r >= ED and (r - ED) in rby:
                            vector.wait_ge(s_f, cum_ext[r - ED])
                        # WAR: bridge-snapshot DMA reads the round-14 slot
                        if r - ED == SNAP_ROUNDS[1]:
                            vector.wait_ge(s_o, 16)
                    vector.wait_ge(s_pd, 2 * (r - 1) + c + 1)
                    nc.vector.tensor_mul(
                        et[r % ED][:, c0:c0 + n],
                        psd[c][:],
                        xsl(r, c0, n)).then_inc(s_vd, 1)

        @block.gpsimd
        def _(gpsimd):
            for r in range(1, L) if NP else ():
                if r == CB[chunk_of[r]]:
                    chunk_arrived(gpsimd, r)
                for c in range(2, 4):
                    c0, n = CHAIN_COLS[c]
                    pc0 = 0 if c == 2 else NP
                    if c == 2 and r - ED == SNAP_ROUNDS[1]:
                        gpsimd.wait_ge(s_o, 16)
                    gpsimd.wait_ge(s_h, 2 * (r - 1) + (c - 2) + 1)
                    nc.gpsimd.tensor_mul(
                        et[r % ED][:, c0:c0 + n],
                        hb[r % 2][:, pc0:pc0 + n],
                        xsl(r, c0, n)).then_inc(s_vp, 1)

    return nc


def _postprocess(snaps, fin, core, c, events, slot_of, inv_core, x0blk,
                 grids_core):
    """Per-core host math (float64): stitch segment offsets, read finals.

    snaps holds device snapshots for SNAP_ROUNDS[1:] = (14, 30, 32); the
    round-0 snapshot is the X init itself (x0blk = xp[:, 0:NCOL]).  Grid-A
    chains pair segment i@0 with segment i-1@32 (i>=2; i==1 pairs with the
    exact segment 0 @30); grid-B chains pair i@0 with i-1@32 (i>=2; i==1
    bridges to the exact segment 0 @14)."""
    ls = np.log(np.maximum(np.concatenate(
        [np.asarray(x0blk)[None], np.asarray(snaps)]
    ).astype(np.float64), 1e-300))  # (4, 128, NCOL)
    # slot s of (seg, b): value vector = snap[:, 64*h:64*h+64, col]
    # stitching: A[s] = A[s-1] + mean_j(d)
    # build per-(seg,b) gather indices from inv map
    slot_idx = np.empty((SEG, BPC), np.int64)
    slot_idx[inv_core[:, 0], inv_core[:, 1]] = np.arange(2 * NCOL)
    h = slot_idx // NCOL
    col = slot_idx % NCOL

    def seg_vals(si, s):
        # (64, BPC) class-vectors of segment s from snapshot si
        hh, cc = h[s], col[s]
        return ls[si, (64 * hh)[None, :] + np.arange(64)[:, None], cc[None, :]]

    A = np.zeros((SEG, BPC))
    gb = np.asarray(grids_core, bool)
    for s in range(1, SEG):
        if s == 1:
            prev_a = seg_vals(2, 0) + SNAP_ROUNDS[2] * c   # exact seg0 @30
            prev_b = seg_vals(1, 0) + SNAP_ROUNDS[1] * c   # exact seg0 @14
            prev = np.where(gb[None, :], prev_b, prev_a)
        else:
            prev = seg_vals(3, s - 1) + SNAP_ROUNDS[3] * c
        cur = seg_vals(0, s)
        d = prev - cur
        A[s] = A[s - 1] + d.mean(axis=0)

    lf = np.log(np.maximum(np.asarray(fin).astype(np.float64), 1e-300))
    res = np.empty(BPC)
    for b in range(BPC):
        e = slot_of[(core, b)]
        r, _, _, s = events[e]
        res[b] = lf[:, e].sum() + 64.0 * (r * c + A[s, b])
    return res


def kernel(pad_x, transitions, origination, batch_sizes):
    from concourse.bass_utils import run_bass_kernel_spmd

    pad_x = np.asarray(pad_x)
    transitions = np.asarray(transitions)
    origination = np.asarray(origination)
    batch_sizes = np.asarray(batch_sizes)

    c = _c_step(transitions, pad_x)
    events, slot_of, grids = _schedule(batch_sizes)
    maps = _slot_maps(events, slot_of)
    xraw, wmat, inv = _build_host_inputs(pad_x, transitions, origination,
                                         c, maps, grids)
    rby = _ranges_by_round(events)

    key = batch_sizes.tobytes()
    if key not in _CACHE:
        _CACHE[key] = _build_program(rby)
    nc = _CACHE[key]

    in_maps = [{"xp": xraw[i], "wm": wmat} for i in range(NCORES)]
    out = run_bass_kernel_spmd(nc, in_maps, list(range(NCORES)))

    res = np.empty(B, np.float32)
    for i in range(NCORES):
        r = _postprocess(out.results[i]["snaps"], out.results[i]["fin"],
                         i, c, events, slot_of, inv[i],
                         xraw[i][:, 0:NCOL], grids[i])
        res[i * BPC:(i + 1) * BPC] = r.astype(np.float32)
    return res



# revision 4
# speedup vs baseline: 1.2201x; 1.2201x over previous
"""Linear-chain CRF forward pass on 8 Trainium2 NeuronCores.

Reference recurrence (per batch element b):
    alpha_t[j] = x_t[j] + logsumexp_k(alpha_{t-1}[k] + trans[j,k])
    out[b] = sum_j alpha_{L_b - 1}[j]

Exp-space device formulation with a constant per-step log shift c folded
into the transition matrix:
    E_t = (Mc @ E_{t-1}) * X_t,  Mc[j,k] = exp(trans[j,k] - c),  X_t = exp(x_t)

The T=2048-step serial chain is cut into segments of SL steps processed in
parallel as independent column-slots, each warmed up W=2 rounds from a raw
X init (Birkhoff contraction converges the direction); per-segment log
offsets are recovered on the host by telescoping class-mean log-ratios at
segment boundaries (round-SL state of segment s-1 vs the host-known round-0
init of segment s).

Structure (vs a dense segment grid):
  - Batch elements are grouped by extraction round then dealt round-robin
    to cores, so every core sees (nearly) the same extraction schedule and
    the same total work.
  - Only segments s <= tstar//SL exist per element: dead slots are never
    shipped, computed, or stored.  Columns are death-sorted so the alive
    set each round is a suffix [d(r), NCOL); matmuls, multiplies and X DMA
    slabs cover only that suffix.
  - Slot classes and death rounds: extraction cols die at max-core r_e;
    grid-B seg0 at the bridge snapshot SL/2-W; everything else at SL (the
    boundary-pairing snapshot; grid-A seg0 read at SL-W from cols at the
    front of the death-SL region).
  - Per round: 2 DVE-direct chains (PE matmul -> PSUM fp32, DVE multiplies
    by X straight out of PSUM) + 2 "z" chains (PE matmul -> PSUM, ACT
    copies PSUM -> SBUF bf16, DVE multiplies all-SBUF bf16 at 2x rate).
    GPSIMD/Pool is unused: its PE->ACT->Pool serial cycle (~1.1us) cannot
    cycle inside sub-us rounds.
  - X = exp(x) is precomputed on the host and shipped bf16 in per-round
    variable-width slabs, chunked through a 4-deep SBUF ring.
"""

from contextlib import ExitStack

import numpy as np

B, T, C = 256, 2048, 64
NCORES = 8
BPC = B // NCORES          # 32
SL = 16                    # steps per segment
W = 2                      # warmup rounds
L = SL + 1                 # rounds 0..SL
SNAPB = SL // 2 - W        # grid-B bridge snapshot round
SNAPA = SL - W             # grid-A seg0 snapshot round
ED = 6                     # et ring depth
DCH = 2                    # rounds per X DMA chunk
RING = 4
NSX = 3
ZC = 260                   # cols per z-chain (2 chains); 0 disables
PRE_FILL = 2
FILLERS = 2

_CACHE = {}


def _c_step(transitions, pad_x):
    """Mean per-step growth of max_j alpha, from a short host simulation."""
    x = np.asarray(pad_x[:4], np.float64)
    tr = np.asarray(transitions, np.float64)
    a = x[:, 0, :]
    tot, n = 0.0, 0
    for t in range(1, 257):
        s = a[:, None, :] + tr[None, :, :]
        m = s.max(axis=2, keepdims=True)
        a_new = x[:, t, :] + np.log(np.exp(s - m).sum(axis=2)) + m[:, :, 0]
        tot += float((a_new.max(axis=1) - a.max(axis=1)).mean())
        n += 1
        a = a_new
    return tot / n


def _elem_sched(ts):
    """(grid_b, ext_seg_index, r_e, seg0_t0_class) for one element."""
    rho = ts % SL
    bthresh = max(SL // 2, SL - 6)
    if ts >= bthresh and rho >= bthresh:
        i = (ts + SL // 2) // SL
        r = ts - SL * i + SL // 2 + W
        return True, i, r
    if ts < SL:
        return False, 0, ts
    return False, ts // SL, rho + W


def _t0_of(grid_b, s):
    """Init timestep of segment s for an element on grid A/B."""
    if s == 0:
        return 0
    return SL * s - (SL // 2 + W if grid_b else W)


class _Plan:
    pass


def _plan(batch_sizes):
    bs = np.asarray(batch_sizes).astype(np.int64)
    p = _Plan()

    # --- assignment: group by r_e, round-robin to cores -------------------
    info = []
    for b in range(B):
        ts = int(bs[b]) - 1
        g, i, r = _elem_sched(ts)
        info.append((r, ts, b, g, i))
    info.sort()
    p.gidx = np.zeros((NCORES, BPC), np.int64)   # [core, e] -> global b
    p.re = np.zeros((NCORES, BPC), np.int64)
    p.tstar = np.zeros((NCORES, BPC), np.int64)
    p.grid = np.zeros((NCORES, BPC), bool)
    p.iseg = np.zeros((NCORES, BPC), np.int64)
    for rank, (r, ts, b, g, i) in enumerate(info):
        k, e = rank % NCORES, rank // NCORES
        p.gidx[k, e] = b
        p.re[k, e] = r
        p.tstar[k, e] = ts
        p.grid[k, e] = g
        p.iseg[k, e] = i
    # within each core, events are already sorted by r_e (global sort)
    ext_death = p.re.max(axis=0)                 # [BPC]
    assert int(ext_death.max()) <= SL

    # --- full-slot counts per core ---------------------------------------
    # classes: B0 (grid-B seg0, death SNAPB), D (everything else, death SL;
    # grid-A seg0 slots go at the FRONT of the D region for the SNAPA dump)
    nb0 = np.zeros(NCORES, np.int64)
    na0 = np.zeros(NCORES, np.int64)
    nmid = np.zeros(NCORES, np.int64)
    for k in range(NCORES):
        for e in range(BPC):
            i = int(p.iseg[k, e])
            if i == 0:
                continue
            if p.grid[k, e]:
                nb0[k] += 1
            else:
                na0[k] += 1
            nmid[k] += i - 1
    NB0 = int(max((int(n) + 1) // 2 for n in nb0))
    NA0 = int(max((int(n) + 1) // 2 for n in na0))
    ND = int(max((int(na0[k] + nmid[k]) + 1) // 2 for k in range(NCORES)))
    ND += 2   # headroom so z/d chain splits have slack
    assert ND >= NA0

    # --- global death-sorted columns -------------------------------------
    cols = [(int(ext_death[e]), 0, e) for e in range(BPC)]       # ext
    cols += [(SNAPB, 1, i) for i in range(NB0)]                  # B0
    cols.sort()
    cols += [(SL, 2, i) for i in range(ND)]                      # D region
    p.ncol = len(cols)
    p.col_death = np.array([cc[0] for cc in cols], np.int64)
    p.ext_col = np.zeros(BPC, np.int64)
    b0_cols = []
    d0 = None
    for ci, (_, cls, ident) in enumerate(cols):
        if cls == 0:
            p.ext_col[ident] = ci
        elif cls == 1:
            b0_cols.append(ci)
        elif d0 is None:
            d0 = ci
    p.b0_cols = np.array(b0_cols, np.int64)
    p.dreg0 = d0                     # start of death-SL region
    assert np.all(np.diff(p.col_death) >= 0)

    p.d = np.array([int(np.searchsorted(p.col_death, r))
                    for r in range(L + 1)], np.int64)

    # --- per-core slot assignment ----------------------------------------
    # slotcol[k, e, s] = column of segment s of element e (s < iseg);
    # slothalf likewise.  ext slot: (ext_col[e], half 0).
    p.slotcol = -np.ones((NCORES, BPC, (T // SL) + 2), np.int64)
    p.slothalf = np.zeros((NCORES, BPC, (T // SL) + 2), np.int64)
    p.na0max = NA0
    for k in range(NCORES):
        # B0 halves: columns b0_cols, halves 0/1
        bslots = [(c, h) for c in b0_cols for h in (0, 1)]
        bi = 0
        # D halves: A0 first (cols d0..), then middles
        dslots = [(d0 + i, h) for h in (0, 1) for i in range(ND)]
        # order: half 0 of all cols, then half 1 -- A0 dump range needs the
        # first na0 slots inside cols [d0, d0+NA0): reorder so A0 fills
        # (d0+i, 0) then (d0+i, 1) column-major within the first NA0 cols.
        a0slots = [(d0 + i, h) for i in range(NA0) for h in (0, 1)]
        restslots = [(d0 + i, h) for i in range(NA0, ND) for h in (0, 1)]
        ai = 0
        rest = iter(a0slots[:] + restslots)
        used = set()
        for e in range(BPC):
            i = int(p.iseg[k, e])
            if i == 0:
                continue
            if p.grid[k, e]:
                c, h = bslots[bi]
                bi += 1
            else:
                c, h = a0slots[ai]
                ai += 1
                used.add((c, h))
            p.slotcol[k, e, 0] = c
            p.slothalf[k, e, 0] = h
        # middles from remaining D halves
        avail = iter([sl_ for sl_ in a0slots + restslots if sl_ not in used])
        for e in range(BPC):
            i = int(p.iseg[k, e])
            for s in range(1, i):
                c, h = next(avail)
                p.slotcol[k, e, s] = c
                p.slothalf[k, e, s] = h

    # --- extraction copy ops ---------------------------------------------
    # rounds where any core extracts event e: [min_k re, max_k re]; copy at
    # each round r the union e-range, split at non-ext columns.
    re_min = p.re.min(axis=0)
    re_max = p.re.max(axis=0)
    p.copies = []       # list of (round, e_a, n, et_col_a, fin_a)
    fin_off = 0
    for r in range(L):
        es = [e for e in range(BPC) if re_min[e] <= r <= re_max[e]]
        if not es:
            continue
        runs = []
        for e in es:
            cc = int(p.ext_col[e])
            if runs and e == runs[-1][1] + 1 and cc == runs[-1][3] + 1:
                runs[-1][1] = e
                runs[-1][3] = cc
            else:
                runs.append([e, e, cc, cc])
        for (e0, e1, c0, _c1) in runs:
            n = e1 - e0 + 1
            p.copies.append((r, e0, n, c0, fin_off))
            fin_off += n
    p.nfin = fin_off
    # fin col for core k, event e: the copy at round re[k, e] covering e
    p.fincol = np.zeros((NCORES, BPC), np.int64)
    for k in range(NCORES):
        for e in range(BPC):
            r = int(p.re[k, e])
            for (rr, e0, n, c0, fa) in p.copies:
                if rr == r and e0 <= e < e0 + n:
                    p.fincol[k, e] = fa + (e - e0)
                    break
            else:
                raise AssertionError("no copy op for event")
    p.cum_copies = np.zeros(L + 1, np.int64)
    for r in range(L):
        p.cum_copies[r + 1] = p.cum_copies[r] + sum(
            1 for (rr, *_x) in p.copies if rr == r)

    # --- chain geometry ---------------------------------------------------
    p.zc = ZC
    zs = p.ncol - 2 * ZC
    assert zs > p.dreg0 + NA0, "z region must stay inside death-SL cols"
    p.zs = zs
    p.mids = np.zeros(L, np.int64)
    for r in range(1, L):
        lo = int(p.d[r])
        m = (lo + zs) // 2
        assert m - lo <= 512 and zs - m <= 512
        p.mids[r] = m
        assert lo <= int(p.ext_col.max()) + 1 or True
    # extraction cols must live in chain 0: max ext col < mid at its round
    for (r, e0, n, c0, fa) in p.copies:
        if r >= 1:
            assert c0 + n <= int(p.mids[r]), "ext cols must be in chain 0"
    assert int(p.b0_cols.max(initial=-1)) < int(p.mids[SNAPB]) \
        if len(p.b0_cols) else True

    # --- DMA chunking -----------------------------------------------------
    p.A = np.array([p.ncol - int(p.d[r]) for r in range(L)], np.int64)
    p.O = np.zeros(L + 1, np.int64)
    p.O[1] = p.ncol                      # slab 0 = rawa covers all cols
    for r in range(1, L):
        p.O[r + 1] = p.O[r] + p.A[r]
    p.ntot = int(p.O[L])
    CB = [1, 2]
    while CB[-1] < L:
        CB.append(min(CB[-1] + DCH, L))
    p.CB = CB
    p.nchunk = len(CB) - 1
    p.chunk_of = [0] * L
    for kk in range(p.nchunk):
        for r in range(CB[kk], CB[kk + 1]):
            p.chunk_of[r] = kk
    p.chunkw = [int(p.O[CB[kk + 1]] - p.O[CB[kk]]) for kk in range(p.nchunk)]
    p.maxchunkw = max(p.chunkw)

    # snapshot ranges (cols)
    p.snapb_rng = (int(p.b0_cols.min()), int(p.b0_cols.max()) + 1) \
        if len(p.b0_cols) else (0, 0)
    p.snapa_rng = (p.dreg0, p.dreg0 + NA0)
    p.snapf_rng = (p.dreg0, p.ncol)
    # early/late fin split: copies with round <= L-5 go early
    p.fin_split = 0
    p.fin_ops_early = 0
    for (r, e0, n, c0, fa) in p.copies:
        if r <= L - 5:
            p.fin_ops_early += 1
            p.fin_split = max(p.fin_split, fa + n)
    if p.fin_split > p.nfin - 4:
        p.fin_split = 0
    return p


def _build_host_inputs(p, pad_x, transitions, origination, c):
    import ml_dtypes
    mc = np.exp(np.asarray(transitions, np.float64) - c).astype(np.float32)
    wmat = np.zeros((128, 128), ml_dtypes.bfloat16)
    wmat[:64, :64] = mc.T.astype(ml_dtypes.bfloat16)
    wmat[64:, 64:] = mc.T.astype(ml_dtypes.bfloat16)

    x0 = np.asarray(pad_x, np.float32)
    x0 = x0.copy()
    x0[:, 0, :] += np.asarray(origination, np.float32)[None, :]

    ncol = p.ncol
    xraw = np.empty((NCORES, 128, p.ntot), ml_dtypes.bfloat16)
    xfull = np.empty((128, L, ncol), np.float32)
    for k in range(NCORES):
        # t0 per (col, half); -10**9 marks dummy (X = 1)
        t0s = np.full((2, ncol), -10 ** 9, np.int64)
        bofs = np.zeros((2, ncol), np.int64)
        for e in range(BPC):
            gb = int(p.gidx[k, e])
            i = int(p.iseg[k, e])
            g = bool(p.grid[k, e])
            # ext slot
            t0s[0, int(p.ext_col[e])] = _t0_of(g, i)
            bofs[0, int(p.ext_col[e])] = gb
            for s in range(0, i):
                cc = int(p.slotcol[k, e, s])
                hh = int(p.slothalf[k, e, s])
                t0s[hh, cc] = _t0_of(g, s)
                bofs[hh, cc] = gb
        for hh in range(2):
            t_idx = t0s[hh][:, None] + np.arange(L)[None, :]   # (ncol, L)
            valid = (t_idx >= 0) & (t_idx < T)
            t_clip = np.clip(t_idx, 0, T - 1)
            blk = x0[bofs[hh][:, None], t_clip, :]             # (ncol, L, C)
            blk = np.where(valid[:, :, None], blk, 0.0)
            eb = np.exp(blk)                                   # (ncol, L, C)
            xfull[64 * hh:64 * hh + 64] = eb.transpose(2, 1, 0)
        # pack slabs: round 0 all cols, round r >= 1 suffix [d(r), ncol)
        out = xraw[k]
        out[:, 0:ncol] = xfull[:, 0, :].astype(ml_dtypes.bfloat16)
        for r in range(1, L):
            o = int(p.O[r])
            d = int(p.d[r])
            out[:, o:o + ncol - d] = \
                xfull[:, r, d:].astype(ml_dtypes.bfloat16)
    return xraw, wmat


def _build_program(p):
    import concourse.bass as bass
    from concourse import mybir

    dt = mybir.dt
    ncol, zc, zs = p.ncol, p.zc, p.zs
    CB = p.CB

    nc = bass.Bass()
    xp = nc.declare_dram_parameter("xp", [128, p.ntot], dt.bfloat16, False)
    wm = nc.declare_dram_parameter("wm", [128, 128], dt.bfloat16, False)
    snapb = nc.declare_dram_parameter(
        "snapb", [128, max(p.snapb_rng[1] - p.snapb_rng[0], 1)],
        dt.bfloat16, True)
    snapa = nc.declare_dram_parameter(
        "snapa", [128, max(p.snapa_rng[1] - p.snapa_rng[0], 1)],
        dt.bfloat16, True)
    snapf = nc.declare_dram_parameter(
        "snapf", [128, p.snapf_rng[1] - p.snapf_rng[0]], dt.bfloat16, True)
    fin = nc.declare_dram_parameter("fin", [64, p.nfin], dt.bfloat16, True)

    nz = 2 if zc else 0

    with ExitStack() as ctx:
        def sb(name, shape, d):
            return ctx.enter_context(nc.sbuf_tensor(name, shape, d))
        w = sb("w", [128, 128], dt.bfloat16)
        rawa = sb("rawa", [128, ncol], dt.bfloat16)
        raw = [sb(f"raw{i}", [128, p.maxchunkw], dt.bfloat16)
               for i in range(RING)]
        et = [sb(f"et{i}", [128, ncol], dt.bfloat16) for i in range(ED)]
        hbz = [sb(f"hbz{i}", [128, max(2 * zc, 2)], dt.bfloat16)
               for i in range(2)]
        fin_t = sb("fin_t", [64, p.nfin], dt.bfloat16)
        psd = [ctx.enter_context(
            nc.psum_tensor(f"psd{cidx}", [128, 512], dt.float32))
            for cidx in range(2)]
        psz = [ctx.enter_context(
            nc.psum_tensor(f"psz{i}", [128, 1024], dt.float32))
            for i in range(2)] if zc else None
        psf = ctx.enter_context(nc.psum_tensor("psf", [128, 128], dt.float32))
        s_w = ctx.enter_context(nc.semaphore("s_w"))
        s_xa = ctx.enter_context(nc.semaphore("s_xa"))
        s_x = tuple(ctx.enter_context(nc.semaphore(f"s_x{i}"))
                    for i in range(NSX))
        s_pd = ctx.enter_context(nc.semaphore("s_pd"))
        s_pz = ctx.enter_context(nc.semaphore("s_pz"))
        s_h = ctx.enter_context(nc.semaphore("s_h"))
        s_vd = ctx.enter_context(nc.semaphore("s_vd"))
        s_vz = ctx.enter_context(nc.semaphore("s_vz"))
        s_f = ctx.enter_context(nc.semaphore("s_f"))
        s_o = ctx.enter_context(nc.semaphore("s_o"))
        block = ctx.enter_context(nc.Block())

        def drng(r):
            lo, m = int(p.d[r]), int(p.mids[r])
            return ((lo, m - lo), (m, zs - m))

        def zrng(cidx):
            return (zs + cidx * zc, zc)

        def xsl(r, c0, n):
            kk = p.chunk_of[r]
            off = int(p.O[r] - p.O[CB[kk]]) + (c0 - int(p.d[r]))
            return raw[kk % RING][:, off:off + n]

        def chunk_arrived(eng, r):
            kk = p.chunk_of[r]
            eng.wait_ge(s_x[kk % NSX], 16 * (kk // NSX + 1))

        def muls_done(eng, r):
            eng.wait_ge(s_vd, 2 * r)
            if nz:
                eng.wait_ge(s_vz, 2 * r)

        @block.sync
        def _(sync):
            sync.dma_start(rawa[:], xp[:, 0:ncol]).then_inc(s_xa, 16)
            for kk in range(p.nchunk):
                if kk >= RING:
                    r_last = CB[kk - RING + 1] - 1
                    sync.wait_ge(s_vd, 2 * r_last)
                    if nz:
                        sync.wait_ge(s_vz, 2 * r_last)
                if kk >= NSX:
                    sync.wait_ge(s_x[kk % NSX], 16 * (kk // NSX))
                o0, o1 = int(p.O[CB[kk]]), int(p.O[CB[kk + 1]])
                sync.dma_start(
                    raw[kk % RING][:, :o1 - o0], xp[:, o0:o1],
                ).then_inc(s_x[kk % NSX], 16)
            # snapshots + fin (chunks for so few rounds finish early here)
            if p.snapb_rng[1] > p.snapb_rng[0]:
                muls_done(sync, SNAPB)
                sync.dma_start(
                    snapb[:],
                    et[SNAPB % ED][:, p.snapb_rng[0]:p.snapb_rng[1]],
                ).then_inc(s_o, 16)
            muls_done(sync, SNAPA)
            sync.dma_start(
                snapa[:],
                et[SNAPA % ED][:, p.snapa_rng[0]:p.snapa_rng[1]],
            ).then_inc(s_o, 16)
            if p.fin_split:
                sync.wait_ge(s_f, p.fin_ops_early)
                sync.dma_start(fin[:, 0:p.fin_split],
                               fin_t[:, 0:p.fin_split]).then_inc(s_o, 16)
                sync.wait_ge(s_f, len(p.copies))
                sync.dma_start(fin[:, p.fin_split:],
                               fin_t[:, p.fin_split:]).then_inc(s_o, 16)
            else:
                sync.wait_ge(s_f, len(p.copies))
                sync.dma_start(fin[:, :], fin_t[:]).then_inc(s_o, 16)
            muls_done(sync, SL)
            sync.dma_start(
                snapf[:],
                et[SL % ED][:, p.snapf_rng[0]:p.snapf_rng[1]],
            ).then_inc(s_o, 16)

        @block.scalar
        def _(scalar):
            nc.scalar.dma_start(w[:], wm[:, :]).then_inc(s_w, 16)
            copies_by_round = {}
            for (r, e0, n, c0, fa) in p.copies:
                copies_by_round.setdefault(r, []).append((e0, n, c0, fa))
            if 0 in copies_by_round:
                scalar.wait_ge(s_xa, 16)
                for (e0, n, c0, fa) in copies_by_round[0]:
                    nc.scalar.copy(fin_t[:, fa:fa + n],
                                   rawa[0:64, c0:c0 + n]).then_inc(s_f, 1)
            for r in range(1, L):
                for cidx in range(nz):
                    z0, _n = zrng(cidx)
                    scalar.wait_ge(s_pz, 2 * (r - 1) + cidx + 1)
                    nc.scalar.copy(
                        hbz[r % 2][:, cidx * zc:cidx * zc + zc],
                        psz[r % 2][:, 512 * cidx:512 * cidx + zc],
                    ).then_inc(s_h, 1)
                if r in copies_by_round:
                    scalar.wait_ge(s_vd, 2 * (r - 1) + 1)
                    for (e0, n, c0, fa) in copies_by_round[r]:
                        nc.scalar.copy(
                            fin_t[:, fa:fa + n],
                            et[r % ED][0:64, c0:c0 + n]).then_inc(s_f, 1)

        @block.tensor
        def _(tensor):
            def filler(n=1):
                for _ in range(n):
                    nc.tensor.matmul(psf[:], w[:], w[:, 0:128],
                                     start=True, stop=True)

            tensor.wait_ge(s_w, 16)
            filler(PRE_FILL)
            for r in range(1, L):
                first = True
                for cidx in range(nz):
                    z0, _n = zrng(cidx)
                    if r == 1:
                        if first:
                            tensor.wait_ge(s_xa, 16)
                            first = False
                        mov = rawa[:, z0:z0 + zc]
                    else:
                        tensor.wait_ge(s_vz, 2 * (r - 1) + cidx - 1)
                        mov = et[(r - 1) % ED][:, z0:z0 + zc]
                    nc.tensor.matmul(
                        psz[r % 2][:, 512 * cidx:512 * cidx + zc],
                        w[:], mov, start=True, stop=True).then_inc(s_pz, 1)
                for cidx, (c0, n) in enumerate(drng(r)):
                    if r == 1:
                        if first:
                            tensor.wait_ge(s_xa, 16)
                            first = False
                        mov = rawa[:, c0:c0 + n]
                    else:
                        tensor.wait_ge(s_vd, 2 * (r - 1) + cidx - 1)
                        mov = et[(r - 1) % ED][:, c0:c0 + n]
                    nc.tensor.matmul(psd[cidx][:, :n], w[:], mov,
                                     start=True, stop=True).then_inc(s_pd, 1)
                filler(FILLERS)

        @block.vector
        def _(vector):
            for r in range(1, L):
                if r == CB[p.chunk_of[r]]:
                    chunk_arrived(vector, r)
                for cidx, (c0, n) in enumerate(drng(r)):
                    if cidx == 0:
                        if r >= ED and p.cum_copies[r - ED + 1] > \
                                p.cum_copies[r - ED]:
                            vector.wait_ge(s_f, int(p.cum_copies[r - ED + 1]))
                        if r - ED == SNAPB and \
                                p.snapb_rng[1] > p.snapb_rng[0]:
                            vector.wait_ge(s_o, 16)
                    vector.wait_ge(s_pd, 2 * (r - 1) + cidx + 1)
                    nc.vector.tensor_mul(
                        et[r % ED][:, c0:c0 + n],
                        psd[cidx][:, :n],
                        xsl(r, c0, n)).then_inc(s_vd, 1)
                for cidx in range(nz):
                    z0, _n = zrng(cidx)
                    vector.wait_ge(s_h, 2 * (r - 1) + cidx + 1)
                    nc.vector.tensor_mul(
                        et[r % ED][:, z0:z0 + zc],
                        hbz[r % 2][:, cidx * zc:cidx * zc + zc],
                        xsl(r, z0, zc)).then_inc(s_vz, 1)

    return nc


def _postprocess(p, k, outs, xraw_k, c):
    """Host math for core k: stitch offsets, read finals (float64)."""
    lx0 = np.log(np.maximum(
        np.asarray(xraw_k[:, 0:p.ncol], np.float64), 1e-300))
    lsb = np.log(np.maximum(np.asarray(outs["snapb"], np.float64), 1e-300))
    lsa = np.log(np.maximum(np.asarray(outs["snapa"], np.float64), 1e-300))
    lsf = np.log(np.maximum(np.asarray(outs["snapf"], np.float64), 1e-300))
    lf = np.log(np.maximum(np.asarray(outs["fin"], np.float64), 1e-300))

    def slot_vec(arr, rng0, col, half):
        return arr[64 * half:64 * half + 64, col - rng0]

    res = np.empty(BPC)
    for e in range(BPC):
        i = int(p.iseg[k, e])
        g = bool(p.grid[k, e])
        r_e = int(p.re[k, e])
        A = 0.0
        for s in range(1, i + 1):
            # cur = round-0 init of segment s (ext slot when s == i)
            if s == i:
                ccur, hcur = int(p.ext_col[e]), 0
            else:
                ccur, hcur = int(p.slotcol[k, e, s]), \
                    int(p.slothalf[k, e, s])
            cur = slot_vec(lx0, 0, ccur, hcur)
            if s == 1:
                c0c, h0c = int(p.slotcol[k, e, 0]), int(p.slothalf[k, e, 0])
                if g:
                    prev = slot_vec(lsb, p.snapb_rng[0], c0c, h0c) \
                        + SNAPB * c
                else:
                    prev = slot_vec(lsa, p.snapa_rng[0], c0c, h0c) \
                        + SNAPA * c
            else:
                cpc, hpc = int(p.slotcol[k, e, s - 1]), \
                    int(p.slothalf[k, e, s - 1])
                prev = slot_vec(lsf, p.snapf_rng[0], cpc, hpc) + SL * c
            A += (prev - cur).mean()
        res[e] = lf[:, int(p.fincol[k, e])].sum() + 64.0 * (r_e * c + A)
    return res


def kernel(pad_x, transitions, origination, batch_sizes):
    from concourse.bass_utils import run_bass_kernel_spmd

    pad_x = np.asarray(pad_x)
    transitions = np.asarray(transitions)
    origination = np.asarray(origination)
    batch_sizes = np.asarray(batch_sizes)

    c = _c_step(transitions, pad_x)
    p = _plan(batch_sizes)
    xraw, wmat = _build_host_inputs(p, pad_x, transitions, origination, c)

    key = batch_sizes.tobytes()
    if key not in _CACHE:
        _CACHE[key] = _build_program(p)
    nc = _CACHE[key]

    in_maps = [{"xp": xraw[i], "wm": wmat} for i in range(NCORES)]
    out = run_bass_kernel_spmd(nc, in_maps, list(range(NCORES)))

    res = np.empty(B, np.float32)
    for k in range(NCORES):
        r = _postprocess(p, k, out.results[k], xraw[k], c)
        for e in range(BPC):
            res[int(p.gidx[k, e])] = np.float32(r[e])
    return res


# revision 16
# speedup vs baseline: 1.4409x; 1.1811x over previous
"""Linear-chain CRF forward pass on 8 Trainium2 NeuronCores.

Reference recurrence (per batch element b):
    alpha_t[j] = x_t[j] + logsumexp_k(alpha_{t-1}[k] + trans[j,k])
    out[b] = sum_j alpha_{L_b - 1}[j]

Exp-space device formulation with a constant per-step log shift c folded
into the transition matrix:
    E_t = (Mc @ E_{t-1}) * X_t,  Mc[j,k] = exp(trans[j,k] - c),  X_t = exp(x_t)

The T=2048-step serial chain is cut into segments of SL steps processed in
parallel as independent column-slots, each warmed up W=2 rounds from a raw
X init (Birkhoff contraction converges the direction); per-segment log
offsets are recovered on the host by telescoping class-mean log-ratios at
segment boundaries (round-SL state of segment s-1 vs the host-known round-0
init of segment s).

Structure (vs a dense segment grid):
  - Batch elements are grouped by extraction round then dealt round-robin
    to cores, so every core sees (nearly) the same extraction schedule and
    the same total work.
  - Only segments s <= tstar//SL exist per element: dead slots are never
    shipped, computed, or stored.  Columns are death-sorted so the alive
    set each round is a suffix [d(r), NCOL); matmuls, multiplies and X DMA
    slabs cover only that suffix.
  - Slot classes and death rounds: extraction cols die at max-core r_e;
    grid-B seg0 at the bridge snapshot SL/2-W; everything else at SL (the
    boundary-pairing snapshot; grid-A seg0 read at SL-W from cols at the
    front of the death-SL region).
  - Per round: 2 DVE-direct chains (PE matmul -> PSUM fp32, DVE multiplies
    by X straight out of PSUM) + 2 "z" chains (PE matmul -> PSUM, ACT
    copies PSUM -> SBUF bf16, DVE multiplies all-SBUF bf16 at 2x rate).
    GPSIMD/Pool is unused: its PE->ACT->Pool serial cycle (~1.1us) cannot
    cycle inside sub-us rounds.
  - X = exp(x) is precomputed on the host and shipped bf16 in per-round
    variable-width slabs, chunked through a 4-deep SBUF ring.
"""

from contextlib import ExitStack

import numpy as np

B, T, C = 256, 2048, 64
NCORES = 8
BPC = B // NCORES          # 32
SL = 16                    # steps per segment
W = 2                      # warmup rounds
L = SL + 1                 # rounds 0..SL
SNAPB = SL // 2 - W        # grid-B bridge snapshot round
SNAPA = SL - W             # grid-A seg0 snapshot round
ED = 6                     # et ring depth
DCH = 2                    # rounds per X DMA chunk
RING = 4
NSX = 3
ZC = 0                     # cols per z-chain (ACT copy + DVE 2x); 0 disables
YP = 190                   # cols per pool chain (ACT copy + Pool mul); x2
PRE_FILL = 2
FILLERS = 2

_CACHE = {}


def _c_step(transitions, pad_x):
    """Mean per-step growth of max_j alpha, from a short host simulation."""
    x = np.asarray(pad_x[:4], np.float64)
    tr = np.asarray(transitions, np.float64)
    a = x[:, 0, :]
    tot, n = 0.0, 0
    for t in range(1, 257):
        s = a[:, None, :] + tr[None, :, :]
        m = s.max(axis=2, keepdims=True)
        a_new = x[:, t, :] + np.log(np.exp(s - m).sum(axis=2)) + m[:, :, 0]
        tot += float((a_new.max(axis=1) - a.max(axis=1)).mean())
        n += 1
        a = a_new
    return tot / n


def _elem_sched(ts):
    """(grid_b, ext_seg_index, r_e, seg0_t0_class) for one element."""
    rho = ts % SL
    bthresh = max(SL // 2, SL - 6)
    if ts >= bthresh and rho >= bthresh:
        i = (ts + SL // 2) // SL
        r = ts - SL * i + SL // 2 + W
        return True, i, r
    if ts < SL:
        return False, 0, ts
    return False, ts // SL, rho + W


def _t0_of(grid_b, s):
    """Init timestep of segment s for an element on grid A/B."""
    if s == 0:
        return 0
    return SL * s - (SL // 2 + W if grid_b else W)


class _Plan:
    pass


def _plan(batch_sizes):
    bs = np.asarray(batch_sizes).astype(np.int64)
    p = _Plan()

    # --- assignment: group by r_e, round-robin to cores -------------------
    info = []
    for b in range(B):
        ts = int(bs[b]) - 1
        g, i, r = _elem_sched(ts)
        info.append((r, ts, b, g, i))
    info.sort()
    p.gidx = np.zeros((NCORES, BPC), np.int64)   # [core, e] -> global b
    p.re = np.zeros((NCORES, BPC), np.int64)
    p.tstar = np.zeros((NCORES, BPC), np.int64)
    p.grid = np.zeros((NCORES, BPC), bool)
    p.iseg = np.zeros((NCORES, BPC), np.int64)
    for rank, (r, ts, b, g, i) in enumerate(info):
        k, e = rank % NCORES, rank // NCORES
        p.gidx[k, e] = b
        p.re[k, e] = r
        p.tstar[k, e] = ts
        p.grid[k, e] = g
        p.iseg[k, e] = i
    # within each core, events are already sorted by r_e (global sort)
    ext_death = p.re.max(axis=0)                 # [BPC]
    assert int(ext_death.max()) <= SL

    # --- full-slot counts per core ---------------------------------------
    # classes: B0 (grid-B seg0, death SNAPB), D (everything else, death SL;
    # grid-A seg0 slots go at the FRONT of the D region for the SNAPA dump)
    nb0 = np.zeros(NCORES, np.int64)
    na0 = np.zeros(NCORES, np.int64)
    nmid = np.zeros(NCORES, np.int64)
    for k in range(NCORES):
        for e in range(BPC):
            i = int(p.iseg[k, e])
            if i == 0:
                continue
            if p.grid[k, e]:
                nb0[k] += 1
            else:
                na0[k] += 1
            nmid[k] += i - 1
    NB0 = int(max((int(n) + 1) // 2 for n in nb0))
    NA0 = int(max((int(n) + 1) // 2 for n in na0))
    ND = int(max((int(na0[k] + nmid[k]) + 1) // 2 for k in range(NCORES)))
    ND += 2   # headroom so z/d chain splits have slack
    assert ND >= NA0

    # --- global death-sorted columns -------------------------------------
    cols = [(int(ext_death[e]), 0, e) for e in range(BPC)]       # ext
    cols += [(SNAPB, 1, i) for i in range(NB0)]                  # B0
    cols.sort()
    cols += [(SL, 2, i) for i in range(ND)]                      # D region
    p.ncol = len(cols)
    p.col_death = np.array([cc[0] for cc in cols], np.int64)
    p.ext_col = np.zeros(BPC, np.int64)
    b0_cols = []
    d0 = None
    for ci, (_, cls, ident) in enumerate(cols):
        if cls == 0:
            p.ext_col[ident] = ci
        elif cls == 1:
            b0_cols.append(ci)
        elif d0 is None:
            d0 = ci
    p.b0_cols = np.array(b0_cols, np.int64)
    p.dreg0 = d0                     # start of death-SL region
    assert np.all(np.diff(p.col_death) >= 0)

    p.d = np.array([int(np.searchsorted(p.col_death, r))
                    for r in range(L + 1)], np.int64)

    # --- per-core slot assignment ----------------------------------------
    # slotcol[k, e, s] = column of segment s of element e (s < iseg);
    # slothalf likewise.  ext slot: (ext_col[e], half 0).
    p.slotcol = -np.ones((NCORES, BPC, (T // SL) + 2), np.int64)
    p.slothalf = np.zeros((NCORES, BPC, (T // SL) + 2), np.int64)
    p.na0max = NA0
    for k in range(NCORES):
        # B0 halves: columns b0_cols, halves 0/1
        bslots = [(c, h) for c in b0_cols for h in (0, 1)]
        bi = 0
        # D halves: A0 first (cols d0..), then middles
        dslots = [(d0 + i, h) for h in (0, 1) for i in range(ND)]
        # order: half 0 of all cols, then half 1 -- A0 dump range needs the
        # first na0 slots inside cols [d0, d0+NA0): reorder so A0 fills
        # (d0+i, 0) then (d0+i, 1) column-major within the first NA0 cols.
        a0slots = [(d0 + i, h) for i in range(NA0) for h in (0, 1)]
        restslots = [(d0 + i, h) for i in range(NA0, ND) for h in (0, 1)]
        ai = 0
        rest = iter(a0slots[:] + restslots)
        used = set()
        for e in range(BPC):
            i = int(p.iseg[k, e])
            if i == 0:
                continue
            if p.grid[k, e]:
                c, h = bslots[bi]
                bi += 1
            else:
                c, h = a0slots[ai]
                ai += 1
                used.add((c, h))
            p.slotcol[k, e, 0] = c
            p.slothalf[k, e, 0] = h
        # middles from remaining D halves
        avail = iter([sl_ for sl_ in a0slots + restslots if sl_ not in used])
        for e in range(BPC):
            i = int(p.iseg[k, e])
            for s in range(1, i):
                c, h = next(avail)
                p.slotcol[k, e, s] = c
                p.slothalf[k, e, s] = h

    # --- extraction copy ops ---------------------------------------------
    # rounds where any core extracts event e: [min_k re, max_k re]; copy at
    # each round r the union e-range, split at non-ext columns.
    re_min = p.re.min(axis=0)
    re_max = p.re.max(axis=0)
    p.copies = []       # list of (round, e_a, n, et_col_a, fin_a)
    fin_off = 0
    for r in range(L):
        es = [e for e in range(BPC) if re_min[e] <= r <= re_max[e]]
        if not es:
            continue
        runs = []
        for e in es:
            cc = int(p.ext_col[e])
            if runs and e == runs[-1][1] + 1 and cc == runs[-1][3] + 1:
                runs[-1][1] = e
                runs[-1][3] = cc
            else:
                runs.append([e, e, cc, cc])
        for (e0, e1, c0, _c1) in runs:
            n = e1 - e0 + 1
            p.copies.append((r, e0, n, c0, fin_off))
            fin_off += n
    p.nfin = fin_off
    # fin col for core k, event e: the copy at round re[k, e] covering e
    p.fincol = np.zeros((NCORES, BPC), np.int64)
    for k in range(NCORES):
        for e in range(BPC):
            r = int(p.re[k, e])
            for (rr, e0, n, c0, fa) in p.copies:
                if rr == r and e0 <= e < e0 + n:
                    p.fincol[k, e] = fa + (e - e0)
                    break
            else:
                raise AssertionError("no copy op for event")
    p.cum_copies = np.zeros(L + 1, np.int64)
    for r in range(L):
        p.cum_copies[r + 1] = p.cum_copies[r] + sum(
            1 for (rr, *_x) in p.copies if rr == r)

    # --- chain geometry ---------------------------------------------------
    p.zc = ZC
    p.yp = YP
    zs = p.ncol - 2 * ZC - 2 * YP
    assert zs > p.dreg0, "hop suffix must stay inside death-SL cols"
    p.zs = zs
    p.mids = np.zeros(L, np.int64)
    for r in range(1, L):
        lo = int(p.d[r])
        m = (lo + zs) // 2
        assert m - lo <= 512 and zs - m <= 512
        p.mids[r] = m
        assert lo <= int(p.ext_col.max()) + 1 or True
    # extraction cols must live in chain 0: max ext col < mid at its round
    for (r, e0, n, c0, fa) in p.copies:
        if r >= 1:
            assert c0 + n <= int(p.mids[r]), "ext cols must be in chain 0"
    assert int(p.b0_cols.max(initial=-1)) < int(p.mids[SNAPB]) \
        if len(p.b0_cols) else True

    # --- DMA chunking -----------------------------------------------------
    p.A = np.array([p.ncol - int(p.d[r]) for r in range(L)], np.int64)
    p.O = np.zeros(L + 1, np.int64)
    p.O[1] = p.ncol                      # slab 0 = rawa covers all cols
    for r in range(1, L):
        p.O[r + 1] = p.O[r] + p.A[r]
    p.ntot = int(p.O[L])
    CB = [1, 2]
    while CB[-1] < L:
        CB.append(min(CB[-1] + DCH, L))
    p.CB = CB
    p.nchunk = len(CB) - 1
    p.chunk_of = [0] * L
    for kk in range(p.nchunk):
        for r in range(CB[kk], CB[kk + 1]):
            p.chunk_of[r] = kk
    p.chunkw = [int(p.O[CB[kk + 1]] - p.O[CB[kk]]) for kk in range(p.nchunk)]
    p.maxchunkw = max(p.chunkw)

    # snapshot ranges (cols)
    p.snapb_rng = (int(p.b0_cols.min()), int(p.b0_cols.max()) + 1) \
        if len(p.b0_cols) else (0, 0)
    p.snapa_rng = (p.dreg0, p.dreg0 + NA0)
    p.snapf_rng = (p.dreg0, p.ncol)
    # early/late fin split: copies with round <= L-5 go early
    p.fin_split = 0
    p.fin_ops_early = 0
    for (r, e0, n, c0, fa) in p.copies:
        if r <= L - 5:
            p.fin_ops_early += 1
            p.fin_split = max(p.fin_split, fa + n)
    if p.fin_split > p.nfin - 4:
        p.fin_split = 0
    return p


def _build_host_inputs(p, pad_x, transitions, origination, c):
    import ml_dtypes
    mc = np.exp(np.asarray(transitions, np.float64) - c).astype(np.float32)
    wmat = np.zeros((128, 128), ml_dtypes.bfloat16)
    wmat[:64, :64] = mc.T.astype(ml_dtypes.bfloat16)
    wmat[64:, 64:] = mc.T.astype(ml_dtypes.bfloat16)

    x0 = np.asarray(pad_x, np.float32)
    x0 = x0.copy()
    x0[:, 0, :] += np.asarray(origination, np.float32)[None, :]

    ncol = p.ncol
    xraw = np.empty((NCORES, 128, p.ntot), ml_dtypes.bfloat16)
    xfull = np.empty((128, L, ncol), np.float32)
    for k in range(NCORES):
        # t0 per (col, half); -10**9 marks dummy (X = 1)
        t0s = np.full((2, ncol), -10 ** 9, np.int64)
        bofs = np.zeros((2, ncol), np.int64)
        for e in range(BPC):
            gb = int(p.gidx[k, e])
            i = int(p.iseg[k, e])
            g = bool(p.grid[k, e])
            # ext slot
            t0s[0, int(p.ext_col[e])] = _t0_of(g, i)
            bofs[0, int(p.ext_col[e])] = gb
            for s in range(0, i):
                cc = int(p.slotcol[k, e, s])
                hh = int(p.slothalf[k, e, s])
                t0s[hh, cc] = _t0_of(g, s)
                bofs[hh, cc] = gb
        for hh in range(2):
            t_idx = t0s[hh][:, None] + np.arange(L)[None, :]   # (ncol, L)
            valid = (t_idx >= 0) & (t_idx < T)
            t_clip = np.clip(t_idx, 0, T - 1)
            blk = x0[bofs[hh][:, None], t_clip, :]             # (ncol, L, C)
            blk = np.where(valid[:, :, None], blk, 0.0)
            eb = np.exp(blk)                                   # (ncol, L, C)
            xfull[64 * hh:64 * hh + 64] = eb.transpose(2, 1, 0)
        # pack slabs: round 0 all cols, round r >= 1 suffix [d(r), ncol)
        out = xraw[k]
        out[:, 0:ncol] = xfull[:, 0, :].astype(ml_dtypes.bfloat16)
        for r in range(1, L):
            o = int(p.O[r])
            d = int(p.d[r])
            out[:, o:o + ncol - d] = \
                xfull[:, r, d:].astype(ml_dtypes.bfloat16)
    return xraw, wmat


def _build_program(p):
    import concourse.bass as bass
    from concourse import mybir

    dt = mybir.dt
    ncol, zc, yp, zs = p.ncol, p.zc, p.yp, p.zs
    CB = p.CB

    nc = bass.Bass()
    xp = nc.declare_dram_parameter("xp", [128, p.ntot], dt.bfloat16, False)
    wm = nc.declare_dram_parameter("wm", [128, 128], dt.bfloat16, False)
    snapb = nc.declare_dram_parameter(
        "snapb", [128, max(p.snapb_rng[1] - p.snapb_rng[0], 1)],
        dt.bfloat16, True)
    snapa = nc.declare_dram_parameter(
        "snapa", [128, max(p.snapa_rng[1] - p.snapa_rng[0], 1)],
        dt.bfloat16, True)
    snapf = nc.declare_dram_parameter(
        "snapf", [128, p.snapf_rng[1] - p.snapf_rng[0]], dt.bfloat16, True)
    fin = nc.declare_dram_parameter("fin", [64, p.nfin], dt.bfloat16, True)

    nz = 2 if zc else 0
    npl = 2 if yp else 0

    with ExitStack() as ctx:
        def sb(name, shape, d):
            return ctx.enter_context(nc.sbuf_tensor(name, shape, d))
        w = sb("w", [128, 128], dt.bfloat16)
        rawa = sb("rawa", [128, ncol], dt.bfloat16)
        raw = [sb(f"raw{i}", [128, p.maxchunkw], dt.bfloat16)
               for i in range(RING)]
        et = [sb(f"et{i}", [128, ncol], dt.bfloat16) for i in range(ED)]
        hbz = [sb(f"hbz{i}", [128, max(2 * zc, 2)], dt.bfloat16)
               for i in range(2)]
        hbp = [sb(f"hbp{i}", [128, max(2 * yp, 2)], dt.bfloat16)
               for i in range(2)]
        fin_t = sb("fin_t", [64, p.nfin], dt.bfloat16)
        psd = [ctx.enter_context(
            nc.psum_tensor(f"psd{cidx}", [128, 512], dt.float32))
            for cidx in range(2)]
        psz = [ctx.enter_context(
            nc.psum_tensor(f"psz{i}", [128, 1024], dt.float32))
            for i in range(2)] if zc else None
        psp = [ctx.enter_context(
            nc.psum_tensor(f"psp{i}", [128, 1024], dt.float32))
            for i in range(2)] if yp else None
        psf = ctx.enter_context(nc.psum_tensor("psf", [128, 128], dt.float32))
        s_w = ctx.enter_context(nc.semaphore("s_w"))
        s_xa = ctx.enter_context(nc.semaphore("s_xa"))
        s_x = tuple(ctx.enter_context(nc.semaphore(f"s_x{i}"))
                    for i in range(NSX))
        s_pd = ctx.enter_context(nc.semaphore("s_pd"))
        s_pz = ctx.enter_context(nc.semaphore("s_pz"))
        s_pp = ctx.enter_context(nc.semaphore("s_pp"))
        s_h = ctx.enter_context(nc.semaphore("s_h"))
        s_hp = ctx.enter_context(nc.semaphore("s_hp"))
        s_vp = ctx.enter_context(nc.semaphore("s_vp"))
        s_vd = ctx.enter_context(nc.semaphore("s_vd"))
        s_vz = ctx.enter_context(nc.semaphore("s_vz"))
        s_f = ctx.enter_context(nc.semaphore("s_f"))
        s_o = ctx.enter_context(nc.semaphore("s_o"))
        block = ctx.enter_context(nc.Block())

        def drng(r):
            lo, m = int(p.d[r]), int(p.mids[r])
            return ((lo, m - lo), (m, zs - m))

        def prng(cidx):
            return (zs + cidx * yp, yp)

        def zrng(cidx):
            return (zs + 2 * yp + cidx * zc, zc)

        def xsl(r, c0, n):
            kk = p.chunk_of[r]
            off = int(p.O[r] - p.O[CB[kk]]) + (c0 - int(p.d[r]))
            return raw[kk % RING][:, off:off + n]

        def chunk_arrived(eng, r):
            kk = p.chunk_of[r]
            eng.wait_ge(s_x[kk % NSX], 16 * (kk // NSX + 1))

        def muls_done(eng, r):
            eng.wait_ge(s_vd, 2 * r)
            if nz:
                eng.wait_ge(s_vz, 2 * r)
            if npl:
                eng.wait_ge(s_vp, 2 * r)

        @block.sync
        def _(sync):
            sync.dma_start(rawa[:], xp[:, 0:ncol]).then_inc(s_xa, 16)
            for kk in range(p.nchunk):
                if kk >= RING:
                    r_last = CB[kk - RING + 1] - 1
                    sync.wait_ge(s_vd, 2 * r_last)
                    if nz:
                        sync.wait_ge(s_vz, 2 * r_last)
                    if npl:
                        sync.wait_ge(s_vp, 2 * r_last)
                if kk >= NSX:
                    sync.wait_ge(s_x[kk % NSX], 16 * (kk // NSX))
                o0, o1 = int(p.O[CB[kk]]), int(p.O[CB[kk + 1]])
                sync.dma_start(
                    raw[kk % RING][:, :o1 - o0], xp[:, o0:o1],
                ).then_inc(s_x[kk % NSX], 16)
            # snapshots + fin (chunks for so few rounds finish early here)
            if p.snapb_rng[1] > p.snapb_rng[0]:
                muls_done(sync, SNAPB)
                sync.dma_start(
                    snapb[:],
                    et[SNAPB % ED][:, p.snapb_rng[0]:p.snapb_rng[1]],
                ).then_inc(s_o, 16)
            muls_done(sync, SNAPA)
            sync.dma_start(
                snapa[:],
                et[SNAPA % ED][:, p.snapa_rng[0]:p.snapa_rng[1]],
            ).then_inc(s_o, 16)
            if p.fin_split:
                sync.wait_ge(s_f, p.fin_ops_early)
                sync.dma_start(fin[:, 0:p.fin_split],
                               fin_t[:, 0:p.fin_split]).then_inc(s_o, 16)
                sync.wait_ge(s_f, len(p.copies))
                sync.dma_start(fin[:, p.fin_split:],
                               fin_t[:, p.fin_split:]).then_inc(s_o, 16)
            else:
                sync.wait_ge(s_f, len(p.copies))
                sync.dma_start(fin[:, :], fin_t[:]).then_inc(s_o, 16)
            muls_done(sync, SL)
            sync.dma_start(
                snapf[:],
                et[SL % ED][:, p.snapf_rng[0]:p.snapf_rng[1]],
            ).then_inc(s_o, 16)

        @block.scalar
        def _(scalar):
            nc.scalar.dma_start(w[:], wm[:, :]).then_inc(s_w, 16)
            copies_by_round = {}
            for (r, e0, n, c0, fa) in p.copies:
                copies_by_round.setdefault(r, []).append((e0, n, c0, fa))
            if 0 in copies_by_round:
                scalar.wait_ge(s_xa, 16)
                for (e0, n, c0, fa) in copies_by_round[0]:
                    nc.scalar.copy(fin_t[:, fa:fa + n],
                                   rawa[0:64, c0:c0 + n]).then_inc(s_f, 1)
            for r in range(1, L):
                for cidx in range(npl):
                    scalar.wait_ge(s_pp, 2 * (r - 1) + cidx + 1)
                    nc.scalar.copy(
                        hbp[r % 2][:, cidx * yp:cidx * yp + yp],
                        psp[r % 2][:, 512 * cidx:512 * cidx + yp],
                    ).then_inc(s_hp, 1)
                for cidx in range(nz):
                    scalar.wait_ge(s_pz, 2 * (r - 1) + cidx + 1)
                    nc.scalar.copy(
                        hbz[r % 2][:, cidx * zc:cidx * zc + zc],
                        psz[r % 2][:, 512 * cidx:512 * cidx + zc],
                    ).then_inc(s_h, 1)
                if r in copies_by_round:
                    scalar.wait_ge(s_vd, 2 * (r - 1) + 1)
                    for (e0, n, c0, fa) in copies_by_round[r]:
                        nc.scalar.copy(
                            fin_t[:, fa:fa + n],
                            et[r % ED][0:64, c0:c0 + n]).then_inc(s_f, 1)

        @block.tensor
        def _(tensor):
            def filler(n=1):
                for _ in range(n):
                    nc.tensor.matmul(psf[:], w[:], w[:, 0:128],
                                     start=True, stop=True)

            tensor.wait_ge(s_w, 16)
            filler(PRE_FILL)
            for r in range(1, L):
                first = True
                for cidx, (c0, n) in enumerate(drng(r)):
                    if r == 1:
                        if first:
                            tensor.wait_ge(s_xa, 16)
                            first = False
                        mov = rawa[:, c0:c0 + n]
                    else:
                        tensor.wait_ge(s_vd, 2 * (r - 1) + cidx - 1)
                        mov = et[(r - 1) % ED][:, c0:c0 + n]
                    nc.tensor.matmul(psd[cidx][:, :n], w[:], mov,
                                     start=True, stop=True).then_inc(s_pd, 1)
                for cidx in range(npl):
                    p0, _n = prng(cidx)
                    if r == 1:
                        mov = rawa[:, p0:p0 + yp]
                    else:
                        tensor.wait_ge(s_vp, 2 * (r - 1) + cidx - 1)
                        mov = et[(r - 1) % ED][:, p0:p0 + yp]
                    nc.tensor.matmul(
                        psp[r % 2][:, 512 * cidx:512 * cidx + yp],
                        w[:], mov, start=True, stop=True).then_inc(s_pp, 1)
                for cidx in range(nz):
                    z0, _n = zrng(cidx)
                    if r == 1:
                        mov = rawa[:, z0:z0 + zc]
                    else:
                        tensor.wait_ge(s_vz, 2 * (r - 1) + cidx - 1)
                        mov = et[(r - 1) % ED][:, z0:z0 + zc]
                    nc.tensor.matmul(
                        psz[r % 2][:, 512 * cidx:512 * cidx + zc],
                        w[:], mov, start=True, stop=True).then_inc(s_pz, 1)
                filler(FILLERS)

        @block.vector
        def _(vector):
            for r in range(1, L):
                if r == CB[p.chunk_of[r]]:
                    chunk_arrived(vector, r)
                for cidx, (c0, n) in enumerate(drng(r)):
                    if cidx == 0:
                        if r >= ED and p.cum_copies[r - ED + 1] > \
                                p.cum_copies[r - ED]:
                            vector.wait_ge(s_f, int(p.cum_copies[r - ED + 1]))
                        if r - ED == SNAPB and \
                                p.snapb_rng[1] > p.snapb_rng[0]:
                            vector.wait_ge(s_o, 16)
                    vector.wait_ge(s_pd, 2 * (r - 1) + cidx + 1)
                    nc.vector.tensor_mul(
                        et[r % ED][:, c0:c0 + n],
                        psd[cidx][:, :n],
                        xsl(r, c0, n)).then_inc(s_vd, 1)
                for cidx in range(nz):
                    z0, _n = zrng(cidx)
                    vector.wait_ge(s_h, 2 * (r - 1) + cidx + 1)
                    nc.vector.tensor_mul(
                        et[r % ED][:, z0:z0 + zc],
                        hbz[r % 2][:, cidx * zc:cidx * zc + zc],
                        xsl(r, z0, zc)).then_inc(s_vz, 1)

        @block.gpsimd
        def _(gpsimd):
            for r in range(1, L) if npl else ():
                if r == CB[p.chunk_of[r]]:
                    chunk_arrived(gpsimd, r)
                for cidx in range(npl):
                    p0, _n = prng(cidx)
                    gpsimd.wait_ge(s_hp, 2 * (r - 1) + cidx + 1)
                    nc.gpsimd.scalar_tensor_tensor(
                        et[r % ED][:, p0:p0 + yp],
                        hbp[r % 2][:, cidx * yp:cidx * yp + yp],
                        1.0,
                        xsl(r, p0, yp),
                        mybir.AluOpType.mult,
                        mybir.AluOpType.mult).then_inc(s_vp, 1)

    return nc


def _postprocess(p, k, outs, xraw_k, c):
    """Host math for core k: stitch offsets, read finals (float64)."""
    lx0 = np.log(np.maximum(
        np.asarray(xraw_k[:, 0:p.ncol], np.float64), 1e-300))
    lsb = np.log(np.maximum(np.asarray(outs["snapb"], np.float64), 1e-300))
    lsa = np.log(np.maximum(np.asarray(outs["snapa"], np.float64), 1e-300))
    lsf = np.log(np.maximum(np.asarray(outs["snapf"], np.float64), 1e-300))
    lf = np.log(np.maximum(np.asarray(outs["fin"], np.float64), 1e-300))

    def slot_vec(arr, rng0, col, half):
        return arr[64 * half:64 * half + 64, col - rng0]

    res = np.empty(BPC)
    for e in range(BPC):
        i = int(p.iseg[k, e])
        g = bool(p.grid[k, e])
        r_e = int(p.re[k, e])
        A = 0.0
        for s in range(1, i + 1):
            # cur = round-0 init of segment s (ext slot when s == i)
            if s == i:
                ccur, hcur = int(p.ext_col[e]), 0
            else:
                ccur, hcur = int(p.slotcol[k, e, s]), \
                    int(p.slothalf[k, e, s])
            cur = slot_vec(lx0, 0, ccur, hcur)
            if s == 1:
                c0c, h0c = int(p.slotcol[k, e, 0]), int(p.slothalf[k, e, 0])
                if g:
                    prev = slot_vec(lsb, p.snapb_rng[0], c0c, h0c) \
                        + SNAPB * c
                else:
                    prev = slot_vec(lsa, p.snapa_rng[0], c0c, h0c) \
                        + SNAPA * c
            else:
                cpc, hpc = int(p.slotcol[k, e, s - 1]), \
                    int(p.slothalf[k, e, s - 1])
                prev = slot_vec(lsf, p.snapf_rng[0], cpc, hpc) + SL * c
            A += (prev - cur).mean()
        res[e] = lf[:, int(p.fincol[k, e])].sum() + 64.0 * (r_e * c + A)
    return res


def kernel(pad_x, transitions, origination, batch_sizes):
    from concourse.bass_utils import run_bass_kernel_spmd

    pad_x = np.asarray(pad_x)
    transitions = np.asarray(transitions)
    origination = np.asarray(origination)
    batch_sizes = np.asarray(batch_sizes)

    c = _c_step(transitions, pad_x)
    p = _plan(batch_sizes)
    xraw, wmat = _build_host_inputs(p, pad_x, transitions, origination, c)

    key = batch_sizes.tobytes()
    if key not in _CACHE:
        _CACHE[key] = _build_program(p)
    nc = _CACHE[key]

    in_maps = [{"xp": xraw[i], "wm": wmat} for i in range(NCORES)]
    out = run_bass_kernel_spmd(nc, in_maps, list(range(NCORES)))

    res = np.empty(B, np.float32)
    for k in range(NCORES):
        r = _postprocess(p, k, out.results[k], xraw[k], c)
        for e in range(BPC):
            res[int(p.gidx[k, e])] = np.float32(r[e])
    return res


# revision 18
# speedup vs baseline: 1.5200x; 1.0548x over previous
"""Linear-chain CRF forward pass on 8 Trainium2 NeuronCores.

Reference recurrence (per batch element b):
    alpha_t[j] = x_t[j] + logsumexp_k(alpha_{t-1}[k] + trans[j,k])
    out[b] = sum_j alpha_{L_b - 1}[j]

Exp-space device formulation with a constant per-step log shift c folded
into the transition matrix:
    E_t = (Mc @ E_{t-1}) * X_t,  Mc[j,k] = exp(trans[j,k] - c),  X_t = exp(x_t)

The T=2048-step serial chain is cut per batch element into a chain of
segments with boundaries on multiples of 8; each segment evolves
independently from a raw X init (warmup W=2 inside the previous segment's
coverage; Birkhoff contraction converges the direction) and the per-segment
log offsets are recovered on the host by telescoping class-mean log-ratios
at the boundaries (each segment's end-state snapshot vs the host-known raw
init of the next segment).

Two segment populations share the 17-round schedule:
  - d-segments (16 steps, 1 step/round) live in the death-sorted front
    region: PE matmul -> PSUM fp32, DVE multiplies by X straight out of
    PSUM (2 chains).  Extraction segments and seg0 are always d-type.
  - hop-segments (8 steps, 1 step per TWO rounds) live in two 512-col
    suffix regions: PE matmul -> PSUM, ACT copies PSUM -> SBUF, GPSIMD
    (Pool) multiplies by X.  Chain A steps on even rounds, chain B on odd
    rounds; the 2-round cadence gives the PE->ACT->Pool serial path two
    full rounds, so it never stalls the d-chains.
  - Only live columns are shipped / computed: columns are death-sorted so
    the alive set each round is a suffix; extraction columns die at their
    extraction round (max 9), seg0 at its boundary snapshot (6 or 14).
  - Batch elements are grouped by extraction round and dealt round-robin
    to cores, so all cores share one extraction schedule and workload.
  - X = exp(x) is precomputed on the host, shipped bf16 in per-round
    variable-width slabs chunked through a 4-deep SBUF ring.
"""

from contextlib import ExitStack

import numpy as np

B, T, C = 256, 2048, 64
NCORES = 8
BPC = B // NCORES          # 32
SL = 16                    # d-segment steps; rounds 0..SL
HS = 8                     # hop-segment steps (one step per 2 rounds)
W = 2                      # warmup rounds
L = SL + 1                 # d rounds 0..16; hop chain B also uses round 17
LB = SL + 2
SNAPB = HS - W             # seg0 snapshot when first gap is 8
SNAPA = SL - W             # seg0 snapshot when first gap is 16
ED = 6                     # et ring depth
DCH = 2                    # rounds per X DMA chunk
RING = 4
NSX = 3
YP = 512                   # cols per hop chain (2 chains)
PRE_FILL = 2
FILLERS = 2

_CACHE = {}


def _c_step(transitions, pad_x):
    """Mean per-step growth of max_j alpha, from a short host simulation."""
    x = np.asarray(pad_x[:4], np.float64)
    tr = np.asarray(transitions, np.float64)
    a = x[:, 0, :]
    tot, n = 0.0, 0
    for t in range(1, 257):
        s = a[:, None, :] + tr[None, :, :]
        m = s.max(axis=2, keepdims=True)
        a_new = x[:, t, :] + np.log(np.exp(s - m).sum(axis=2)) + m[:, :, 0]
        tot += float((a_new.max(axis=1) - a.max(axis=1)).mean())
        n += 1
        a = a_new
    return tot / n


class _Plan:
    pass


def _plan(batch_sizes):
    bs = np.asarray(batch_sizes).astype(np.int64)
    p = _Plan()

    # --- assignment: group by r_e, round-robin to cores -------------------
    info = []
    for b in range(B):
        ts = int(bs[b]) - 1
        if ts < HS:
            r_e, bm = ts, 0
        else:
            bm = (ts // HS) * HS
            r_e = ts - bm + W
        info.append((r_e, ts, b, bm))
    info.sort()
    p.gidx = np.zeros((NCORES, BPC), np.int64)
    p.re = np.zeros((NCORES, BPC), np.int64)
    p.tstar = np.zeros((NCORES, BPC), np.int64)
    p.bm = np.zeros((NCORES, BPC), np.int64)
    for rank, (r_e, ts, b, bm) in enumerate(info):
        k, e = rank % NCORES, rank // NCORES
        p.gidx[k, e] = b
        p.re[k, e] = r_e
        p.tstar[k, e] = ts
        p.bm[k, e] = bm
    ext_death = p.re.max(axis=0)
    assert int(ext_death.max()) <= HS + W

    # --- per-element segment gap lists (8s and 16s), hop quota ------------
    QHOP = 4 * YP              # hop half-slots per core (2 chains x 2 halves)
    p.gaps = [[None] * BPC for _ in range(NCORES)]
    counts = {"B0": np.zeros(NCORES, np.int64),
              "A0": np.zeros(NCORES, np.int64),
              "D": np.zeros(NCORES, np.int64),
              "H": np.zeros(NCORES, np.int64)}
    for k in range(NCORES):
        q = QHOP
        # big elements first so quota parity always resolves
        order = sorted(range(BPC), key=lambda e: -int(p.bm[k, e]))
        for e in order:
            G = int(p.bm[k, e]) // HS
            # seg0's gap is free (B0/A0 column); only mid 8-gaps use hop
            # slots, so up to q+1 eights fit.  Parity: n8 must match G.
            n8 = min(G, q + 1)
            if (n8 - G) % 2:
                n8 -= 1
            if n8 < G % 2:
                n8 = G % 2
            q -= max(n8 - 1, 0)
            n16 = (G - n8) // 2
            # gap list in chain order: one 8 first if any (seg0 -> B0)
            if n8 >= 1:
                gaps = [8] * n8 + [16] * n16
            else:
                gaps = [16] * n16
            assert sum(gaps) == int(p.bm[k, e])
            p.gaps[k][e] = gaps
            if gaps:
                counts["B0" if gaps[0] == 8 else "A0"][k] += 1
                counts["H"][k] += (n8 - 1) if n8 >= 1 else 0
                counts["D"][k] += n16 if n8 >= 1 else n16 - 1
        assert q >= -1
    NB0 = int(max((int(n) + 1) // 2 for n in counts["B0"]))
    NA0 = int(max((int(n) + 1) // 2 for n in counts["A0"]))
    NDM = int(max((int(n) + 1) // 2 for n in counts["D"])) + 1

    # --- global death-sorted d-region columns -----------------------------
    cols = [(int(ext_death[e]), 0, e) for e in range(BPC)]
    cols += [(SNAPB, 1, i) for i in range(NB0)]
    cols.sort()
    cols += [(SNAPA, 2, i) for i in range(NA0)]
    cols += [(SL, 3, i) for i in range(NDM)]
    p.zs = len(cols)
    p.ncol = p.zs + 2 * YP
    p.col_death = np.array([cc[0] for cc in cols], np.int64)
    assert np.all(np.diff(p.col_death) >= 0)
    p.ext_col = np.zeros(BPC, np.int64)
    b0_cols, a0_cols = [], []
    dm0 = None
    for ci, (_, cls, ident) in enumerate(cols):
        if cls == 0:
            p.ext_col[ident] = ci
        elif cls == 1:
            b0_cols.append(ci)
        elif cls == 2:
            a0_cols.append(ci)
        elif dm0 is None:
            dm0 = ci
    p.b0_rng = (b0_cols[0], b0_cols[-1] + 1) if b0_cols else (0, 0)
    p.a0_rng = (a0_cols[0], a0_cols[-1] + 1) if a0_cols else (0, 0)
    p.dm0 = dm0 if dm0 is not None else p.zs

    p.d = np.array([int(np.searchsorted(p.col_death, r))
                    for r in range(L + 1)], np.int64)

    # --- per-core slot assignment ----------------------------------------
    # segs[k][e] = list of (t0, steps, kind, col, half); ext is separate
    p.segs = [[None] * BPC for _ in range(NCORES)]
    for k in range(NCORES):
        it_b0 = iter([(c, h) for c in b0_cols for h in (0, 1)])
        it_a0 = iter([(c, h) for c in a0_cols for h in (0, 1)])
        it_d = iter([(c, h) for c in range(p.dm0, p.zs) for h in (0, 1)])
        it_h = iter([(c, h) for c in range(p.zs, p.ncol) for h in (0, 1)])
        for e in range(BPC):
            gaps = p.gaps[k][e]
            segs = []
            b_cum = 0
            for j, g in enumerate(gaps):
                t0 = 0 if j == 0 else b_cum - W
                if j == 0:
                    kind = "B0" if g == 8 else "A0"
                    col, half = next(it_b0 if g == 8 else it_a0)
                    steps = SNAPB if g == 8 else SNAPA
                elif g == 16:
                    kind, (col, half), steps = "D", next(it_d), SL
                else:
                    kind, (col, half), steps = "H", next(it_h), HS
                segs.append((t0, steps, kind, col, half))
                b_cum += g
            p.segs[k][e] = segs

    # --- extraction copy ops ---------------------------------------------
    re_min = p.re.min(axis=0)
    re_max = p.re.max(axis=0)
    p.copies = []
    fin_off = 0
    for r in range(L):
        es = [e for e in range(BPC) if re_min[e] <= r <= re_max[e]]
        if not es:
            continue
        runs = []
        for e in es:
            cc = int(p.ext_col[e])
            if runs and e == runs[-1][1] + 1 and cc == runs[-1][3] + 1:
                runs[-1][1] = e
                runs[-1][3] = cc
            else:
                runs.append([e, e, cc, cc])
        for (e0, e1, c0, _c1) in runs:
            n = e1 - e0 + 1
            p.copies.append((r, e0, n, c0, fin_off))
            fin_off += n
    p.nfin = fin_off
    p.fincol = np.zeros((NCORES, BPC), np.int64)
    for k in range(NCORES):
        for e in range(BPC):
            r = int(p.re[k, e])
            for (rr, e0, n, c0, fa) in p.copies:
                if rr == r and e0 <= e < e0 + n:
                    p.fincol[k, e] = fa + (e - e0)
                    break
            else:
                raise AssertionError("no copy op for event")
    p.cum_copies = np.zeros(L + 1, np.int64)
    for r in range(L):
        p.cum_copies[r + 1] = p.cum_copies[r] + sum(
            1 for (rr, *_x) in p.copies if rr == r)

    # --- chain geometry ---------------------------------------------------
    p.mids = np.zeros(L, np.int64)
    for r in range(1, L):
        lo = int(p.d[r])
        m = (lo + p.zs) // 2
        assert m - lo <= 512 and p.zs - m <= 512
        p.mids[r] = m
    for (r, e0, n, c0, fa) in p.copies:
        if r >= 1:
            assert c0 + n <= int(p.mids[r])
    assert p.b0_rng[1] <= int(p.mids[SNAPB]) or p.b0_rng[1] == 0

    # --- X slab layout ----------------------------------------------------
    # round r slab: d-part [d(r), zs) for r in 1..SL; hop part (512) for
    # the active chain: A on even r >= 2, B on odd r >= 3 (and r = 17).
    def hop_chain(r):
        if r >= 2 and r % 2 == 0:
            return 0
        if r >= 3 and r % 2 == 1:
            return 1
        return None
    p.hop_chain = hop_chain
    p.O = np.zeros(LB + 1, np.int64)
    p.O[1] = p.ncol                      # rawa
    for r in range(1, LB):
        wdt = (p.zs - int(p.d[r])) if r <= SL else 0
        if hop_chain(r) is not None:
            wdt += YP
        p.O[r + 1] = p.O[r] + wdt
    p.ntot = int(p.O[LB])
    CB = [1, 2]
    while CB[-1] < LB:
        CB.append(min(CB[-1] + DCH, LB))
    p.CB = CB
    p.nchunk = len(CB) - 1
    p.chunk_of = [0] * LB
    for kk in range(p.nchunk):
        for r in range(CB[kk], CB[kk + 1]):
            p.chunk_of[r] = kk
    p.maxchunkw = max(int(p.O[CB[kk + 1]] - p.O[CB[kk]])
                      for kk in range(p.nchunk))

    # early/late fin split
    p.fin_split = 0
    p.fin_ops_early = 0
    for (r, e0, n, c0, fa) in p.copies:
        if r <= HS:
            p.fin_ops_early += 1
            p.fin_split = max(p.fin_split, fa + n)
    if p.fin_split > p.nfin - 4:
        p.fin_split = 0
    return p


def _build_host_inputs(p, pad_x, transitions, origination, c):
    import ml_dtypes
    mc = np.exp(np.asarray(transitions, np.float64) - c).astype(np.float32)
    wmat = np.zeros((128, 128), ml_dtypes.bfloat16)
    wmat[:64, :64] = mc.T.astype(ml_dtypes.bfloat16)
    wmat[64:, 64:] = mc.T.astype(ml_dtypes.bfloat16)

    x0 = np.asarray(pad_x, np.float32).copy()
    x0[:, 0, :] += np.asarray(origination, np.float32)[None, :]

    ncol, zs = p.ncol, p.zs
    xraw = np.empty((NCORES, 128, p.ntot), ml_dtypes.bfloat16)
    for k in range(NCORES):
        t0s = np.full((2, ncol), -10 ** 9, np.int64)
        bofs = np.zeros((2, ncol), np.int64)
        for e in range(BPC):
            gb = int(p.gidx[k, e])
            ts = int(p.tstar[k, e])
            t0s[0, int(p.ext_col[e])] = 0 if ts < HS else int(p.bm[k, e]) - W
            bofs[0, int(p.ext_col[e])] = gb
            for (t0, steps, kind, col, half) in p.segs[k][e]:
                t0s[half, col] = t0
                bofs[half, col] = gb
        # device steps per column: d-region cols see step r at round r;
        # hop cols see step s at round 2s (A) / 2s+1 (B).
        xfull = np.empty((128, L, ncol), np.float32)
        for hh in range(2):
            t_idx = t0s[hh][:, None] + np.arange(L)[None, :]
            valid = (t_idx >= 0) & (t_idx < T) & (t0s[hh][:, None] > -10**8)
            t_clip = np.clip(t_idx, 0, T - 1)
            blk = x0[bofs[hh][:, None], t_clip, :]
            blk = np.where(valid[:, :, None], blk, 0.0)
            xfull[64 * hh:64 * hh + 64] = np.exp(blk).transpose(2, 1, 0)
        out = xraw[k]
        out[:, 0:ncol] = xfull[:, 0, :].astype(ml_dtypes.bfloat16)
        for r in range(1, LB):
            o = int(p.O[r])
            if r <= SL:
                d = int(p.d[r])
                out[:, o:o + zs - d] = \
                    xfull[:, r, d:zs].astype(ml_dtypes.bfloat16)
                o += zs - d
            hc = p.hop_chain(r)
            if hc is not None:
                s = r // 2 if hc == 0 else (r - 1) // 2
                cb = zs + hc * YP
                out[:, o:o + YP] = \
                    xfull[:, s, cb:cb + YP].astype(ml_dtypes.bfloat16)
    return xraw, wmat


def _build_program(p):
    import concourse.bass as bass
    from concourse import mybir

    dt = mybir.dt
    ncol, zs = p.ncol, p.zs
    CB = p.CB

    nc = bass.Bass()
    xp = nc.declare_dram_parameter("xp", [128, p.ntot], dt.bfloat16, False)
    wm = nc.declare_dram_parameter("wm", [128, 128], dt.bfloat16, False)
    snapb = nc.declare_dram_parameter(
        "snapb", [128, max(p.b0_rng[1] - p.b0_rng[0], 1)], dt.bfloat16, True)
    snapa = nc.declare_dram_parameter(
        "snapa", [128, max(p.a0_rng[1] - p.a0_rng[0], 1)], dt.bfloat16, True)
    snapf = nc.declare_dram_parameter(
        "snapf", [128, zs + YP - p.dm0], dt.bfloat16, True)
    snapfb = nc.declare_dram_parameter(
        "snapfb", [128, YP], dt.bfloat16, True)
    fin = nc.declare_dram_parameter("fin", [64, p.nfin], dt.bfloat16, True)

    with ExitStack() as ctx:
        def sb(name, shape, d):
            return ctx.enter_context(nc.sbuf_tensor(name, shape, d))
        w = sb("w", [128, 128], dt.bfloat16)
        rawa = sb("rawa", [128, ncol], dt.bfloat16)
        raw = [sb(f"raw{i}", [128, p.maxchunkw], dt.bfloat16)
               for i in range(RING)]
        et = [sb(f"et{i}", [128, ncol], dt.bfloat16) for i in range(ED)]
        hbp = [sb(f"hbp{i}", [128, YP], dt.bfloat16) for i in range(2)]
        fin_t = sb("fin_t", [64, p.nfin], dt.bfloat16)
        psd = [ctx.enter_context(
            nc.psum_tensor(f"psd{cidx}", [128, 512], dt.float32))
            for cidx in range(2)]
        psp = [ctx.enter_context(
            nc.psum_tensor(f"psp{i}", [128, 512], dt.float32))
            for i in range(2)]
        psf = ctx.enter_context(nc.psum_tensor("psf", [128, 128], dt.float32))
        s_w = ctx.enter_context(nc.semaphore("s_w"))
        s_xa = ctx.enter_context(nc.semaphore("s_xa"))
        s_x = tuple(ctx.enter_context(nc.semaphore(f"s_x{i}"))
                    for i in range(NSX))
        s_pd = ctx.enter_context(nc.semaphore("s_pd"))
        s_pp = ctx.enter_context(nc.semaphore("s_pp"))
        s_hp = ctx.enter_context(nc.semaphore("s_hp"))
        s_vd = ctx.enter_context(nc.semaphore("s_vd"))
        s_vp = ctx.enter_context(nc.semaphore("s_vp"))
        s_f = ctx.enter_context(nc.semaphore("s_f"))
        s_o = ctx.enter_context(nc.semaphore("s_o"))
        block = ctx.enter_context(nc.Block())

        def drng(r):
            lo, m = int(p.d[r]), int(p.mids[r])
            return ((lo, m - lo), (m, zs - m))

        def xsl_d(r, c0, n):
            kk = p.chunk_of[r]
            off = int(p.O[r] - p.O[CB[kk]]) + (c0 - int(p.d[r]))
            return raw[kk % RING][:, off:off + n]

        def xsl_h(r, hc):
            kk = p.chunk_of[r]
            off = int(p.O[r] - p.O[CB[kk]]) + \
                ((zs - int(p.d[r])) if r <= SL else 0)
            return raw[kk % RING][:, off:off + YP]

        def chunk_arrived(eng, r):
            kk = p.chunk_of[r]
            eng.wait_ge(s_x[kk % NSX], 16 * (kk // NSX + 1))

        @block.sync
        def _(sync):
            sync.dma_start(rawa[:], xp[:, 0:ncol]).then_inc(s_xa, 16)
            for kk in range(p.nchunk):
                if kk >= RING:
                    r_last = CB[kk - RING + 1] - 1
                    sync.wait_ge(s_vd, 2 * min(r_last, SL))
                    if r_last >= 2:
                        sync.wait_ge(s_vp, r_last - 1)
                if kk >= NSX:
                    sync.wait_ge(s_x[kk % NSX], 16 * (kk // NSX))
                o0, o1 = int(p.O[CB[kk]]), int(p.O[CB[kk + 1]])
                sync.dma_start(
                    raw[kk % RING][:, :o1 - o0], xp[:, o0:o1],
                ).then_inc(s_x[kk % NSX], 16)
            if p.b0_rng[1] > p.b0_rng[0]:
                sync.wait_ge(s_vd, 2 * SNAPB)
                sync.dma_start(
                    snapb[:],
                    et[SNAPB % ED][:, p.b0_rng[0]:p.b0_rng[1]],
                ).then_inc(s_o, 16)
            if p.a0_rng[1] > p.a0_rng[0]:
                sync.wait_ge(s_vd, 2 * SNAPA)
                sync.dma_start(
                    snapa[:],
                    et[SNAPA % ED][:, p.a0_rng[0]:p.a0_rng[1]],
                ).then_inc(s_o, 16)
            if p.fin_split:
                sync.wait_ge(s_f, p.fin_ops_early)
                sync.dma_start(fin[:, 0:p.fin_split],
                               fin_t[:, 0:p.fin_split]).then_inc(s_o, 16)
                sync.wait_ge(s_f, len(p.copies))
                sync.dma_start(fin[:, p.fin_split:],
                               fin_t[:, p.fin_split:]).then_inc(s_o, 16)
            else:
                sync.wait_ge(s_f, len(p.copies))
                sync.dma_start(fin[:, :], fin_t[:]).then_inc(s_o, 16)
            # merged d + hop-A end snapshot (both land in et[SL % ED])
            sync.wait_ge(s_vd, 2 * SL)
            sync.wait_ge(s_vp, SL - 1)
            sync.dma_start(
                snapf[:], et[SL % ED][:, p.dm0:zs + YP]).then_inc(s_o, 16)
            sync.wait_ge(s_vp, SL)
            sync.dma_start(
                snapfb[:], et[(SL + 1) % ED][:, zs + YP:ncol]).then_inc(s_o, 16)

        @block.scalar
        def _(scalar):
            nc.scalar.dma_start(w[:], wm[:, :]).then_inc(s_w, 16)
            copies_by_round = {}
            for (r, e0, n, c0, fa) in p.copies:
                copies_by_round.setdefault(r, []).append((e0, n, c0, fa))
            if 0 in copies_by_round:
                scalar.wait_ge(s_xa, 16)
                for (e0, n, c0, fa) in copies_by_round[0]:
                    nc.scalar.copy(fin_t[:, fa:fa + n],
                                   rawa[0:64, c0:c0 + n]).then_inc(s_f, 1)
            for r in range(1, LB):
                hc = p.hop_chain(r)
                if hc is not None:
                    scalar.wait_ge(s_pp, r - 1)
                    nc.scalar.copy(hbp[hc][:],
                                   psp[hc][:]).then_inc(s_hp, 1)
                if r in copies_by_round:
                    scalar.wait_ge(s_vd, 2 * (r - 1) + 1)
                    for (e0, n, c0, fa) in copies_by_round[r]:
                        nc.scalar.copy(
                            fin_t[:, fa:fa + n],
                            et[r % ED][0:64, c0:c0 + n]).then_inc(s_f, 1)

        @block.tensor
        def _(tensor):
            def filler(n=1):
                for _ in range(n):
                    nc.tensor.matmul(psf[:], w[:], w[:, 0:128],
                                     start=True, stop=True)

            tensor.wait_ge(s_w, 16)
            filler(PRE_FILL)
            for r in range(1, LB):
                if r <= SL:
                    for cidx, (c0, n) in enumerate(drng(r)):
                        if r == 1:
                            if cidx == 0:
                                tensor.wait_ge(s_xa, 16)
                            mov = rawa[:, c0:c0 + n]
                        else:
                            tensor.wait_ge(s_vd, 2 * (r - 1) + cidx - 1)
                            mov = et[(r - 1) % ED][:, c0:c0 + n]
                        nc.tensor.matmul(
                            psd[cidx][:, :n], w[:], mov,
                            start=True, stop=True).then_inc(s_pd, 1)
                hc = p.hop_chain(r)
                if hc is not None:
                    cb = zs + hc * YP
                    if r <= 3:
                        mov = rawa[:, cb:cb + YP]
                    else:
                        tensor.wait_ge(s_vp, r - 3)
                        mov = et[(r - 2) % ED][:, cb:cb + YP]
                    nc.tensor.matmul(psp[hc][:], w[:], mov,
                                     start=True, stop=True).then_inc(s_pp, 1)
                filler(FILLERS)

        @block.vector
        def _(vector):
            for r in range(1, L):
                if r == CB[p.chunk_of[r]]:
                    chunk_arrived(vector, r)
                for cidx, (c0, n) in enumerate(drng(r)):
                    if cidx == 0:
                        if r >= ED and p.cum_copies[r - ED + 1] > \
                                p.cum_copies[r - ED]:
                            vector.wait_ge(s_f, int(p.cum_copies[r - ED + 1]))
                        if r - ED == SNAPB and p.b0_rng[1] > p.b0_rng[0]:
                            vector.wait_ge(s_o, 16)
                    vector.wait_ge(s_pd, 2 * (r - 1) + cidx + 1)
                    nc.vector.tensor_mul(
                        et[r % ED][:, c0:c0 + n],
                        psd[cidx][:, :n],
                        xsl_d(r, c0, n)).then_inc(s_vd, 1)

        @block.gpsimd
        def _(gpsimd):
            for r in range(2, LB):
                hc = p.hop_chain(r)
                if hc is None:
                    continue
                if p.chunk_of[r] != p.chunk_of[r - 1] or r == 2:
                    chunk_arrived(gpsimd, r)
                cb = zs + hc * YP
                gpsimd.wait_ge(s_hp, r - 1)
                nc.gpsimd.scalar_tensor_tensor(
                    et[r % ED][:, cb:cb + YP],
                    hbp[hc][:],
                    1.0,
                    xsl_h(r, hc),
                    mybir.AluOpType.mult,
                    mybir.AluOpType.mult).then_inc(s_vp, 1)

    return nc


def _postprocess(p, k, outs, xraw_k, c):
    """Host math for core k: stitch offsets, read finals (float64)."""
    lx0 = np.log(np.maximum(
        np.asarray(xraw_k[:, 0:p.ncol], np.float64), 1e-300))
    lsb = np.log(np.maximum(np.asarray(outs["snapb"], np.float64), 1e-300))
    lsa = np.log(np.maximum(np.asarray(outs["snapa"], np.float64), 1e-300))
    lsf = np.log(np.maximum(np.asarray(outs["snapf"], np.float64), 1e-300))
    lsfb = np.log(np.maximum(np.asarray(outs["snapfb"], np.float64), 1e-300))
    lf = np.log(np.maximum(np.asarray(outs["fin"], np.float64), 1e-300))

    def vec(arr, rng0, col, half):
        return arr[64 * half:64 * half + 64, col - rng0]

    res = np.empty(BPC)
    for e in range(BPC):
        segs = p.segs[k][e]
        r_e = int(p.re[k, e])
        A = 0.0
        for j in range(len(segs)):
            t0, steps, kind, col, half = segs[j]
            if kind == "B0":
                prev = vec(lsb, p.b0_rng[0], col, half) + SNAPB * c
            elif kind == "A0":
                prev = vec(lsa, p.a0_rng[0], col, half) + SNAPA * c
            elif kind == "D":
                prev = vec(lsf, p.dm0, col, half) + SL * c
            elif col < p.zs + YP:
                prev = vec(lsf, p.dm0, col, half) + HS * c
            else:
                prev = vec(lsfb, p.zs + YP, col, half) + HS * c
            # cur = raw init of the NEXT segment (or the extraction segment)
            if j + 1 < len(segs):
                ncol_, nhalf = segs[j + 1][3], segs[j + 1][4]
            else:
                ncol_, nhalf = int(p.ext_col[e]), 0
            cur = vec(lx0, 0, ncol_, nhalf)
            A += (prev - cur).mean()
        res[e] = lf[:, int(p.fincol[k, e])].sum() + 64.0 * (r_e * c + A)
    return res


def kernel(pad_x, transitions, origination, batch_sizes):
    from concourse.bass_utils import run_bass_kernel_spmd

    pad_x = np.asarray(pad_x)
    transitions = np.asarray(transitions)
    origination = np.asarray(origination)
    batch_sizes = np.asarray(batch_sizes)

    c = _c_step(transitions, pad_x)
    p = _plan(batch_sizes)
    xraw, wmat = _build_host_inputs(p, pad_x, transitions, origination, c)

    key = batch_sizes.tobytes()
    if key not in _CACHE:
        _CACHE[key] = _build_program(p)
    nc = _CACHE[key]

    in_maps = [{"xp": xraw[i], "wm": wmat} for i in range(NCORES)]
    out = run_bass_kernel_spmd(nc, in_maps, list(range(NCORES)))

    res = np.empty(B, np.float32)
    for k in range(NCORES):
        r = _postprocess(p, k, out.results[k], xraw[k], c)
        for e in range(BPC):
            res[int(p.gidx[k, e])] = np.float32(r[e])
    return res


# revision 27
# speedup vs baseline: 1.5336x; 1.0090x over previous
"""Linear-chain CRF forward pass on 8 Trainium2 NeuronCores.

Reference recurrence (per batch element b):
    alpha_t[j] = x_t[j] + logsumexp_k(alpha_{t-1}[k] + trans[j,k])
    out[b] = sum_j alpha_{L_b - 1}[j]

Exp-space device formulation with a constant per-step log shift c folded
into the transition matrix:
    E_t = (Mc @ E_{t-1}) * X_t,  Mc[j,k] = exp(trans[j,k] - c),  X_t = exp(x_t)

The T=2048-step serial chain is cut per batch element into a chain of
segments with boundaries on multiples of 8; each segment evolves
independently from a raw X init (warmup W=2 inside the previous segment's
coverage; Birkhoff contraction converges the direction) and the per-segment
log offsets are recovered on the host by telescoping class-mean log-ratios
at the boundaries (each segment's end-state snapshot vs the host-known raw
init of the next segment).

Two segment populations share the 17-round schedule:
  - d-segments (16 steps, 1 step/round) live in the death-sorted front
    region: PE matmul -> PSUM fp32, DVE multiplies by X straight out of
    PSUM (2 chains).  Extraction segments and seg0 are always d-type.
  - hop-segments (8 steps, 1 step per TWO rounds) live in two 512-col
    suffix regions: PE matmul -> PSUM, ACT copies PSUM -> SBUF, GPSIMD
    (Pool) multiplies by X.  Chain A steps on even rounds, chain B on odd
    rounds; the 2-round cadence gives the PE->ACT->Pool serial path two
    full rounds, so it never stalls the d-chains.
  - Only live columns are shipped / computed: columns are death-sorted so
    the alive set each round is a suffix; extraction columns die at their
    extraction round (max 9), seg0 at its boundary snapshot (6 or 14).
  - Batch elements are grouped by extraction round and dealt round-robin
    to cores, so all cores share one extraction schedule and workload.
  - X = exp(x) is precomputed on the host, shipped bf16 in per-round
    variable-width slabs chunked through a 4-deep SBUF ring.
"""

from contextlib import ExitStack

import numpy as np

B, T, C = 256, 2048, 64
NCORES = 8
BPC = B // NCORES          # 32
SL = 16                    # d-segment steps; rounds 0..SL
HS = 8                     # hop-segment steps (one step per 2 rounds)
W = 2                      # warmup rounds
L = SL + 1                 # rounds 0..16 (hop chain A even, B odd)
LB = SL + 1
SNAPB = HS - W             # seg0 snapshot when first gap is 8
SNAPA = SL - W             # seg0 snapshot when first gap is 16
ED = 6                     # et ring depth
DCH = 2                    # rounds per X DMA chunk
RING = 4
NSX = 3
YP = 512                   # cols per hop chain (2 chains)
PRE_FILL = 5
FILLERS = 2

_CACHE = {}


def _c_step(transitions, pad_x):
    """Mean per-step growth of max_j alpha, from a short host simulation."""
    x = np.asarray(pad_x[:4], np.float64)
    tr = np.asarray(transitions, np.float64)
    a = x[:, 0, :]
    tot, n = 0.0, 0
    for t in range(1, 257):
        s = a[:, None, :] + tr[None, :, :]
        m = s.max(axis=2, keepdims=True)
        a_new = x[:, t, :] + np.log(np.exp(s - m).sum(axis=2)) + m[:, :, 0]
        tot += float((a_new.max(axis=1) - a.max(axis=1)).mean())
        n += 1
        a = a_new
    return tot / n


class _Plan:
    pass


def _plan(batch_sizes):
    bs = np.asarray(batch_sizes).astype(np.int64)
    p = _Plan()

    # --- assignment: group by r_e, round-robin to cores -------------------
    info = []
    for b in range(B):
        ts = int(bs[b]) - 1
        if ts < HS:
            r_e, bm = ts, 0
        else:
            bm = (ts // HS) * HS
            r_e = ts - bm + W
        info.append((r_e, ts, b, bm))
    info.sort()
    p.gidx = np.zeros((NCORES, BPC), np.int64)
    p.re = np.zeros((NCORES, BPC), np.int64)
    p.tstar = np.zeros((NCORES, BPC), np.int64)
    p.bm = np.zeros((NCORES, BPC), np.int64)
    for rank, (r_e, ts, b, bm) in enumerate(info):
        k, e = rank % NCORES, rank // NCORES
        p.gidx[k, e] = b
        p.re[k, e] = r_e
        p.tstar[k, e] = ts
        p.bm[k, e] = bm
    ext_death = p.re.max(axis=0)
    assert int(ext_death.max()) <= HS + W

    # --- per-element segment gap lists (8s and 16s), hop quota ------------
    QHOP = 4 * YP              # hop half-slots per core (2 chains x 2 halves)
    p.gaps = [[None] * BPC for _ in range(NCORES)]
    counts = {"B0": np.zeros(NCORES, np.int64),
              "A0": np.zeros(NCORES, np.int64),
              "D": np.zeros(NCORES, np.int64),
              "H": np.zeros(NCORES, np.int64)}
    for k in range(NCORES):
        q = QHOP
        # big elements first so quota parity always resolves
        order = sorted(range(BPC), key=lambda e: -int(p.bm[k, e]))
        for e in order:
            G = int(p.bm[k, e]) // HS
            # seg0's gap is free (B0/A0 column); only mid 8-gaps use hop
            # slots, so up to q+1 eights fit.  Parity: n8 must match G.
            n8 = min(G, q + 1)
            if (n8 - G) % 2:
                n8 -= 1
            if n8 < G % 2:
                n8 = G % 2
            q -= max(n8 - 1, 0)
            n16 = (G - n8) // 2
            # gap list in chain order: one 8 first if any (seg0 -> B0)
            if n8 >= 1:
                gaps = [8] * n8 + [16] * n16
            else:
                gaps = [16] * n16
            assert sum(gaps) == int(p.bm[k, e])
            p.gaps[k][e] = gaps
            if gaps:
                counts["B0" if gaps[0] == 8 else "A0"][k] += 1
                counts["H"][k] += (n8 - 1) if n8 >= 1 else 0
                counts["D"][k] += n16 if n8 >= 1 else n16 - 1
        assert q >= -1
    NB0 = int(max((int(n) + 1) // 2 for n in counts["B0"]))
    NA0 = int(max((int(n) + 1) // 2 for n in counts["A0"]))
    NDM = int(max((int(n) + 1) // 2 for n in counts["D"])) + 1

    # --- global death-sorted d-region columns -----------------------------
    cols = [(int(ext_death[e]), 0, e) for e in range(BPC)]
    cols += [(SNAPB, 1, i) for i in range(NB0)]
    cols.sort()
    cols += [(SNAPA, 2, i) for i in range(NA0)]
    cols += [(SL, 3, i) for i in range(NDM)]
    p.zs = len(cols)
    p.ncol = p.zs + 2 * YP
    p.col_death = np.array([cc[0] for cc in cols], np.int64)
    assert np.all(np.diff(p.col_death) >= 0)
    p.ext_col = np.zeros(BPC, np.int64)
    b0_cols, a0_cols = [], []
    dm0 = None
    for ci, (_, cls, ident) in enumerate(cols):
        if cls == 0:
            p.ext_col[ident] = ci
        elif cls == 1:
            b0_cols.append(ci)
        elif cls == 2:
            a0_cols.append(ci)
        elif dm0 is None:
            dm0 = ci
    p.b0_rng = (b0_cols[0], b0_cols[-1] + 1) if b0_cols else (0, 0)
    p.a0_rng = (a0_cols[0], a0_cols[-1] + 1) if a0_cols else (0, 0)
    p.dm0 = dm0 if dm0 is not None else p.zs

    p.d = np.array([int(np.searchsorted(p.col_death, r))
                    for r in range(L + 1)], np.int64)

    # --- per-core slot assignment ----------------------------------------
    # segs[k][e] = list of (t0, steps, kind, col, half); ext is separate
    p.segs = [[None] * BPC for _ in range(NCORES)]
    for k in range(NCORES):
        it_b0 = iter([(c, h) for c in b0_cols for h in (0, 1)])
        it_a0 = iter([(c, h) for c in a0_cols for h in (0, 1)])
        it_d = iter([(c, h) for c in range(p.dm0, p.zs) for h in (0, 1)])
        it_h = iter([(c, h) for c in range(p.zs, p.ncol) for h in (0, 1)])
        for e in range(BPC):
            gaps = p.gaps[k][e]
            segs = []
            b_cum = 0
            for j, g in enumerate(gaps):
                t0 = 0 if j == 0 else b_cum - W
                if j == 0:
                    kind = "B0" if g == 8 else "A0"
                    col, half = next(it_b0 if g == 8 else it_a0)
                    steps = SNAPB if g == 8 else SNAPA
                elif g == 16:
                    kind, (col, half), steps = "D", next(it_d), SL
                else:
                    kind, (col, half), steps = "H", next(it_h), HS
                segs.append((t0, steps, kind, col, half))
                b_cum += g
            p.segs[k][e] = segs

    # --- extraction copy ops ---------------------------------------------
    re_min = p.re.min(axis=0)
    re_max = p.re.max(axis=0)
    p.copies = []
    fin_off = 0
    for r in range(L):
        es = [e for e in range(BPC) if re_min[e] <= r <= re_max[e]]
        if not es:
            continue
        runs = []
        for e in es:
            cc = int(p.ext_col[e])
            if runs and e == runs[-1][1] + 1 and cc == runs[-1][3] + 1:
                runs[-1][1] = e
                runs[-1][3] = cc
            else:
                runs.append([e, e, cc, cc])
        for (e0, e1, c0, _c1) in runs:
            n = e1 - e0 + 1
            p.copies.append((r, e0, n, c0, fin_off))
            fin_off += n
    p.nfin = fin_off
    p.fincol = np.zeros((NCORES, BPC), np.int64)
    for k in range(NCORES):
        for e in range(BPC):
            r = int(p.re[k, e])
            for (rr, e0, n, c0, fa) in p.copies:
                if rr == r and e0 <= e < e0 + n:
                    p.fincol[k, e] = fa + (e - e0)
                    break
            else:
                raise AssertionError("no copy op for event")
    p.cum_copies = np.zeros(L + 1, np.int64)
    for r in range(L):
        p.cum_copies[r + 1] = p.cum_copies[r] + sum(
            1 for (rr, *_x) in p.copies if rr == r)

    # --- chain geometry ---------------------------------------------------
    p.mids = np.zeros(L, np.int64)
    for r in range(1, L):
        lo = int(p.d[r])
        m = (lo + p.zs) // 2
        assert m - lo <= 512 and p.zs - m <= 512
        p.mids[r] = m
    for (r, e0, n, c0, fa) in p.copies:
        if r >= 1:
            assert c0 + n <= int(p.mids[r])
    assert p.b0_rng[1] <= int(p.mids[SNAPB]) or p.b0_rng[1] == 0

    # --- X slab layout ----------------------------------------------------
    # round r slab: d-part [d(r), zs) + hop part (512 cols) for the active
    # chain: A steps on even rounds (2..16, step r/2), B on odd (1..15,
    # step (r+1)/2).  B's step 8 lands at round 15 so its end snapshot
    # ships during round 16.
    def hop_chain(r):
        if r >= 2 and r % 2 == 0:
            return 0
        if r >= 1 and r % 2 == 1:
            return 1
        return None
    p.hop_chain = hop_chain
    p.O = np.zeros(LB + 1, np.int64)
    p.O[1] = p.ncol                      # rawa
    for r in range(1, LB):
        wdt = p.zs - int(p.d[r])
        if hop_chain(r) is not None:
            wdt += YP
        p.O[r + 1] = p.O[r] + wdt
    p.ntot = int(p.O[LB])
    CB = [1, 2]
    while CB[-1] < LB:
        CB.append(min(CB[-1] + DCH, LB))
    p.CB = CB
    p.nchunk = len(CB) - 1
    p.chunk_of = [0] * LB
    for kk in range(p.nchunk):
        for r in range(CB[kk], CB[kk + 1]):
            p.chunk_of[r] = kk
    p.maxchunkw = max(int(p.O[CB[kk + 1]] - p.O[CB[kk]])
                      for kk in range(p.nchunk))

    # early/late fin split
    p.fin_split = 0
    p.fin_ops_early = 0
    for (r, e0, n, c0, fa) in p.copies:
        if r <= HS:
            p.fin_ops_early += 1
            p.fin_split = max(p.fin_split, fa + n)
    if p.fin_split > p.nfin - 4:
        p.fin_split = 0
    return p


def _build_host_inputs(p, pad_x, transitions, origination, c):
    import ml_dtypes
    mc = np.exp(np.asarray(transitions, np.float64) - c).astype(np.float32)
    wmat = np.zeros((128, 128), ml_dtypes.bfloat16)
    wmat[:64, :64] = mc.T.astype(ml_dtypes.bfloat16)
    wmat[64:, 64:] = mc.T.astype(ml_dtypes.bfloat16)

    x0 = np.asarray(pad_x, np.float32).copy()
    x0[:, 0, :] += np.asarray(origination, np.float32)[None, :]

    ncol, zs = p.ncol, p.zs
    xraw = np.empty((NCORES, 128, p.ntot), ml_dtypes.bfloat16)
    for k in range(NCORES):
        t0s = np.full((2, ncol), -10 ** 9, np.int64)
        bofs = np.zeros((2, ncol), np.int64)
        for e in range(BPC):
            gb = int(p.gidx[k, e])
            ts = int(p.tstar[k, e])
            t0s[0, int(p.ext_col[e])] = 0 if ts < HS else int(p.bm[k, e]) - W
            bofs[0, int(p.ext_col[e])] = gb
            for (t0, steps, kind, col, half) in p.segs[k][e]:
                t0s[half, col] = t0
                bofs[half, col] = gb
        # device steps per column: d-region cols see step r at round r;
        # hop cols see step s at round 2s (A) / 2s+1 (B).
        xfull = np.empty((128, L, ncol), np.float32)
        for hh in range(2):
            t_idx = t0s[hh][:, None] + np.arange(L)[None, :]
            valid = (t_idx >= 0) & (t_idx < T) & (t0s[hh][:, None] > -10**8)
            t_clip = np.clip(t_idx, 0, T - 1)
            blk = x0[bofs[hh][:, None], t_clip, :]
            blk = np.where(valid[:, :, None], blk, 0.0)
            xfull[64 * hh:64 * hh + 64] = np.exp(blk).transpose(2, 1, 0)
        out = xraw[k]
        out[:, 0:ncol] = xfull[:, 0, :].astype(ml_dtypes.bfloat16)
        for r in range(1, LB):
            o = int(p.O[r])
            d = int(p.d[r])
            out[:, o:o + zs - d] = \
                xfull[:, r, d:zs].astype(ml_dtypes.bfloat16)
            o += zs - d
            hc = p.hop_chain(r)
            if hc is not None:
                s = r // 2 if hc == 0 else (r + 1) // 2
                cb = zs + hc * YP
                out[:, o:o + YP] = \
                    xfull[:, s, cb:cb + YP].astype(ml_dtypes.bfloat16)
    return xraw, wmat


def _build_program(p):
    import concourse.bass as bass
    from concourse import mybir

    dt = mybir.dt
    ncol, zs = p.ncol, p.zs
    CB = p.CB

    nc = bass.Bass()
    xp = nc.declare_dram_parameter("xp", [128, p.ntot], dt.bfloat16, False)
    wm = nc.declare_dram_parameter("wm", [128, 128], dt.bfloat16, False)
    snapb = nc.declare_dram_parameter(
        "snapb", [128, max(p.b0_rng[1] - p.b0_rng[0], 1)], dt.bfloat16, True)
    snapa = nc.declare_dram_parameter(
        "snapa", [128, max(p.a0_rng[1] - p.a0_rng[0], 1)], dt.bfloat16, True)
    snapf = nc.declare_dram_parameter(
        "snapf", [128, zs + YP - p.dm0], dt.bfloat16, True)
    snapfb = nc.declare_dram_parameter(
        "snapfb", [128, YP], dt.bfloat16, True)
    fin = nc.declare_dram_parameter("fin", [64, p.nfin], dt.bfloat16, True)

    with ExitStack() as ctx:
        def sb(name, shape, d):
            return ctx.enter_context(nc.sbuf_tensor(name, shape, d))
        w = sb("w", [128, 128], dt.bfloat16)
        rawa = sb("rawa", [128, ncol], dt.bfloat16)
        raw = [sb(f"raw{i}", [128, p.maxchunkw], dt.bfloat16)
               for i in range(RING)]
        et = [sb(f"et{i}", [128, ncol], dt.bfloat16) for i in range(ED)]
        hbp = [sb(f"hbp{i}", [128, YP], dt.bfloat16) for i in range(2)]
        fin_t = sb("fin_t", [64, p.nfin], dt.bfloat16)
        psd = [ctx.enter_context(
            nc.psum_tensor(f"psd{cidx}", [128, 512], dt.float32))
            for cidx in range(2)]
        psp = [ctx.enter_context(
            nc.psum_tensor(f"psp{i}", [128, 512], dt.float32))
            for i in range(2)]
        psf = ctx.enter_context(nc.psum_tensor("psf", [128, 128], dt.float32))
        s_w = ctx.enter_context(nc.semaphore("s_w"))
        s_xa = ctx.enter_context(nc.semaphore("s_xa"))
        s_x = tuple(ctx.enter_context(nc.semaphore(f"s_x{i}"))
                    for i in range(NSX))
        s_pd = ctx.enter_context(nc.semaphore("s_pd"))
        s_pp = ctx.enter_context(nc.semaphore("s_pp"))
        s_hp = ctx.enter_context(nc.semaphore("s_hp"))
        s_vd = ctx.enter_context(nc.semaphore("s_vd"))
        s_vp = ctx.enter_context(nc.semaphore("s_vp"))
        s_f = ctx.enter_context(nc.semaphore("s_f"))
        s_o = ctx.enter_context(nc.semaphore("s_o"))
        block = ctx.enter_context(nc.Block())

        def drng(r):
            lo, m = int(p.d[r]), int(p.mids[r])
            return ((lo, m - lo), (m, zs - m))

        def xsl_d(r, c0, n):
            kk = p.chunk_of[r]
            off = int(p.O[r] - p.O[CB[kk]]) + (c0 - int(p.d[r]))
            return raw[kk % RING][:, off:off + n]

        def xsl_h(r, hc):
            kk = p.chunk_of[r]
            off = int(p.O[r] - p.O[CB[kk]]) + \
                ((zs - int(p.d[r])) if r <= SL else 0)
            return raw[kk % RING][:, off:off + YP]

        def chunk_arrived(eng, r):
            kk = p.chunk_of[r]
            eng.wait_ge(s_x[kk % NSX], 16 * (kk // NSX + 1))

        @block.sync
        def _(sync):
            sync.dma_start(w[:], wm[:, :]).then_inc(s_w, 16)
            sync.dma_start(rawa[:, 0:zs], xp[:, 0:zs]).then_inc(s_xa, 16)
            sync.dma_start(rawa[:, zs:ncol],
                           xp[:, zs:ncol]).then_inc(s_xa, 16)
            for kk in range(p.nchunk):
                if kk >= RING:
                    r_last = CB[kk - RING + 1] - 1
                    sync.wait_ge(s_vd, 2 * min(r_last, SL))
                    sync.wait_ge(s_vp, r_last)
                if kk >= NSX:
                    sync.wait_ge(s_x[kk % NSX], 16 * (kk // NSX))
                o0, o1 = int(p.O[CB[kk]]), int(p.O[CB[kk + 1]])
                sync.dma_start(
                    raw[kk % RING][:, :o1 - o0], xp[:, o0:o1],
                ).then_inc(s_x[kk % NSX], 16)
            if p.b0_rng[1] > p.b0_rng[0]:
                sync.wait_ge(s_vd, 2 * SNAPB)
                sync.dma_start(
                    snapb[:],
                    et[SNAPB % ED][:, p.b0_rng[0]:p.b0_rng[1]],
                ).then_inc(s_o, 16)
            if p.a0_rng[1] > p.a0_rng[0]:
                sync.wait_ge(s_vd, 2 * SNAPA)
                sync.dma_start(
                    snapa[:],
                    et[SNAPA % ED][:, p.a0_rng[0]:p.a0_rng[1]],
                ).then_inc(s_o, 16)
            if p.fin_split:
                sync.wait_ge(s_f, p.fin_ops_early)
                sync.dma_start(fin[:, 0:p.fin_split],
                               fin_t[:, 0:p.fin_split]).then_inc(s_o, 16)
                sync.wait_ge(s_f, len(p.copies))
                sync.dma_start(fin[:, p.fin_split:],
                               fin_t[:, p.fin_split:]).then_inc(s_o, 16)
            else:
                sync.wait_ge(s_f, len(p.copies))
                sync.dma_start(fin[:, :], fin_t[:]).then_inc(s_o, 16)
            # hop-B end snapshot (step 8 at round 15) ships during round 16
            sync.wait_ge(s_vp, SL - 1)
            sync.dma_start(
                snapfb[:], et[(SL - 1) % ED][:, zs + YP:ncol]).then_inc(s_o, 16)
            # merged d + hop-A end snapshot (both land in et[SL % ED])
            sync.wait_ge(s_vd, 2 * SL)
            sync.wait_ge(s_vp, SL)
            sync.dma_start(
                snapf[:], et[SL % ED][:, p.dm0:zs + YP]).then_inc(s_o, 16)

        @block.scalar
        def _(scalar):
            copies_by_round = {}
            for (r, e0, n, c0, fa) in p.copies:
                copies_by_round.setdefault(r, []).append((e0, n, c0, fa))
            if 0 in copies_by_round:
                scalar.wait_ge(s_xa, 16)
                for (e0, n, c0, fa) in copies_by_round[0]:
                    nc.scalar.copy(fin_t[:, fa:fa + n],
                                   rawa[0:64, c0:c0 + n]).then_inc(s_f, 1)
            for r in range(1, LB):
                hc = p.hop_chain(r)
                if hc is not None:
                    scalar.wait_ge(s_pp, r)
                    nc.scalar.copy(hbp[hc][:],
                                   psp[hc][:]).then_inc(s_hp, 1)
                if r in copies_by_round:
                    scalar.wait_ge(s_vd, 2 * (r - 1) + 1)
                    for (e0, n, c0, fa) in copies_by_round[r]:
                        nc.scalar.copy(
                            fin_t[:, fa:fa + n],
                            et[r % ED][0:64, c0:c0 + n]).then_inc(s_f, 1)

        @block.tensor
        def _(tensor):
            def filler(n=1):
                for _ in range(n):
                    nc.tensor.matmul(psf[:], w[:], w[:, 0:128],
                                     start=True, stop=True)

            tensor.wait_ge(s_w, 16)
            filler(PRE_FILL)
            for r in range(1, LB):
                for cidx, (c0, n) in enumerate(drng(r)):
                    if r == 1:
                        if cidx == 0:
                            tensor.wait_ge(s_xa, 16)
                        mov = rawa[:, c0:c0 + n]
                    else:
                        tensor.wait_ge(s_vd, 2 * (r - 1) + cidx - 1)
                        mov = et[(r - 1) % ED][:, c0:c0 + n]
                    nc.tensor.matmul(
                        psd[cidx][:, :n], w[:], mov,
                        start=True, stop=True).then_inc(s_pd, 1)
                hc = p.hop_chain(r)
                if hc is not None:
                    cb = zs + hc * YP
                    if r <= 2:
                        if r == 1:
                            tensor.wait_ge(s_xa, 32)
                        mov = rawa[:, cb:cb + YP]
                    else:
                        tensor.wait_ge(s_vp, r - 2)
                        mov = et[(r - 2) % ED][:, cb:cb + YP]
                    nc.tensor.matmul(psp[hc][:], w[:], mov,
                                     start=True, stop=True).then_inc(s_pp, 1)
                filler(FILLERS)

        @block.vector
        def _(vector):
            for r in range(1, L):
                if r == CB[p.chunk_of[r]]:
                    chunk_arrived(vector, r)
                for cidx, (c0, n) in enumerate(drng(r)):
                    if cidx == 0:
                        if r >= ED and p.cum_copies[r - ED + 1] > \
                                p.cum_copies[r - ED]:
                            vector.wait_ge(s_f, int(p.cum_copies[r - ED + 1]))
                        if r - ED == SNAPB and p.b0_rng[1] > p.b0_rng[0]:
                            vector.wait_ge(s_o, 16)
                    vector.wait_ge(s_pd, 2 * (r - 1) + cidx + 1)
                    nc.vector.tensor_mul(
                        et[r % ED][:, c0:c0 + n],
                        psd[cidx][:, :n],
                        xsl_d(r, c0, n)).then_inc(s_vd, 1)

        @block.gpsimd
        def _(gpsimd):
            for r in range(1, LB):
                hc = p.hop_chain(r)
                if hc is None:
                    continue
                if r == 1 or p.chunk_of[r] != p.chunk_of[r - 1]:
                    chunk_arrived(gpsimd, r)
                cb = zs + hc * YP
                gpsimd.wait_ge(s_hp, r)
                nc.gpsimd.scalar_tensor_tensor(
                    et[r % ED][:, cb:cb + YP],
                    hbp[hc][:],
                    1.0,
                    xsl_h(r, hc),
                    mybir.AluOpType.mult,
                    mybir.AluOpType.mult).then_inc(s_vp, 1)

    return nc


def _postprocess(p, k, outs, xraw_k, c):
    """Host math for core k: stitch offsets, read finals (float64)."""
    lx0 = np.log(np.maximum(
        np.asarray(xraw_k[:, 0:p.ncol], np.float64), 1e-300))
    lsb = np.log(np.maximum(np.asarray(outs["snapb"], np.float64), 1e-300))
    lsa = np.log(np.maximum(np.asarray(outs["snapa"], np.float64), 1e-300))
    lsf = np.log(np.maximum(np.asarray(outs["snapf"], np.float64), 1e-300))
    lsfb = np.log(np.maximum(np.asarray(outs["snapfb"], np.float64), 1e-300))
    lf = np.log(np.maximum(np.asarray(outs["fin"], np.float64), 1e-300))

    def vec(arr, rng0, col, half):
        return arr[64 * half:64 * half + 64, col - rng0]

    res = np.empty(BPC)
    for e in range(BPC):
        segs = p.segs[k][e]
        r_e = int(p.re[k, e])
        A = 0.0
        for j in range(len(segs)):
            t0, steps, kind, col, half = segs[j]
            if kind == "B0":
                prev = vec(lsb, p.b0_rng[0], col, half) + SNAPB * c
            elif kind == "A0":
                prev = vec(lsa, p.a0_rng[0], col, half) + SNAPA * c
            elif kind == "D":
                prev = vec(lsf, p.dm0, col, half) + SL * c
            elif col < p.zs + YP:
                prev = vec(lsf, p.dm0, col, half) + HS * c
            else:
                prev = vec(lsfb, p.zs + YP, col, half) + HS * c
            # cur = raw init of the NEXT segment (or the extraction segment)
            if j + 1 < len(segs):
                ncol_, nhalf = segs[j + 1][3], segs[j + 1][4]
            else:
                ncol_, nhalf = int(p.ext_col[e]), 0
            cur = vec(lx0, 0, ncol_, nhalf)
            A += (prev - cur).mean()
        res[e] = lf[:, int(p.fincol[k, e])].sum() + 64.0 * (r_e * c + A)
    return res


def kernel(pad_x, transitions, origination, batch_sizes):
    from concourse.bass_utils import run_bass_kernel_spmd

    pad_x = np.asarray(pad_x)
    transitions = np.asarray(transitions)
    origination = np.asarray(origination)
    batch_sizes = np.asarray(batch_sizes)

    c = _c_step(transitions, pad_x)
    p = _plan(batch_sizes)
    xraw, wmat = _build_host_inputs(p, pad_x, transitions, origination, c)

    key = batch_sizes.tobytes()
    if key not in _CACHE:
        _CACHE[key] = _build_program(p)
    nc = _CACHE[key]

    in_maps = [{"xp": xraw[i], "wm": wmat} for i in range(NCORES)]
    out = run_bass_kernel_spmd(nc, in_maps, list(range(NCORES)))

    res = np.empty(B, np.float32)
    for k in range(NCORES):
        r = _postprocess(p, k, out.results[k], xraw[k], c)
        for e in range(BPC):
            res[int(p.gidx[k, e])] = np.float32(r[e])
    return res


# revision 34
# speedup vs baseline: 1.5571x; 1.0153x over previous
"""Linear-chain CRF forward pass on 8 Trainium2 NeuronCores.

Reference recurrence (per batch element b):
    alpha_t[j] = x_t[j] + logsumexp_k(alpha_{t-1}[k] + trans[j,k])
    out[b] = sum_j alpha_{L_b - 1}[j]

Exp-space device formulation with a constant per-step log shift c folded
into the transition matrix:
    E_t = (Mc @ E_{t-1}) * X_t,  Mc[j,k] = exp(trans[j,k] - c),  X_t = exp(x_t)

The T=2048-step serial chain is cut per batch element into a chain of
segments with boundaries on multiples of 8; each segment evolves
independently from a raw X init (warmup W=2 inside the previous segment's
coverage; Birkhoff contraction converges the direction) and the per-segment
log offsets are recovered on the host by telescoping class-mean log-ratios
at the boundaries (each segment's end-state snapshot vs the host-known raw
init of the next segment).

Two segment populations share the 17-round schedule:
  - d-segments (16 steps, 1 step/round) live in the death-sorted front
    region: PE matmul -> PSUM fp32, DVE multiplies by X straight out of
    PSUM (2 chains).  Extraction segments and seg0 are always d-type.
  - hop-segments (8 steps, 1 step per TWO rounds) live in two 512-col
    suffix regions: PE matmul -> PSUM, ACT copies PSUM -> SBUF, GPSIMD
    (Pool) multiplies by X.  Chain A steps on even rounds, chain B on odd
    rounds; the 2-round cadence gives the PE->ACT->Pool serial path two
    full rounds, so it never stalls the d-chains.
  - Only live columns are shipped / computed: columns are death-sorted so
    the alive set each round is a suffix; extraction columns die at their
    extraction round (max 9), seg0 at its boundary snapshot (6 or 14).
  - Batch elements are grouped by extraction round and dealt round-robin
    to cores, so all cores share one extraction schedule and workload.
  - X = exp(x) is precomputed on the host, shipped bf16 in per-round
    variable-width slabs chunked through a 4-deep SBUF ring.
"""

from contextlib import ExitStack

import numpy as np

B, T, C = 256, 2048, 64
NCORES = 8
BPC = B // NCORES          # 32
SL = 16                    # d-segment steps; rounds 0..SL
HS = 8                     # hop-segment steps (one step per 2 rounds)
W = 2                      # warmup rounds
L = SL + 1                 # rounds 0..16 (hop chain A even, B odd)
LB = SL + 1
SNAPB = HS - W             # seg0 snapshot when first gap is 8
SNAPA = SL - W             # seg0 snapshot when first gap is 16
ED = 6                     # et ring depth
DCH = 2                    # rounds per X DMA chunk
RING = 4
NSX = 3
YP = 440                   # cols per hop chain (2 chains)
PRE_FILL = 5
FILLERS = 2

_CACHE = {}


def _c_step(transitions, pad_x):
    """Mean per-step growth of max_j alpha, from a short host simulation."""
    x = np.asarray(pad_x[:4], np.float64)
    tr = np.asarray(transitions, np.float64)
    a = x[:, 0, :]
    tot, n = 0.0, 0
    for t in range(1, 257):
        s = a[:, None, :] + tr[None, :, :]
        m = s.max(axis=2, keepdims=True)
        a_new = x[:, t, :] + np.log(np.exp(s - m).sum(axis=2)) + m[:, :, 0]
        tot += float((a_new.max(axis=1) - a.max(axis=1)).mean())
        n += 1
        a = a_new
    return tot / n


class _Plan:
    pass


def _plan(batch_sizes):
    bs = np.asarray(batch_sizes).astype(np.int64)
    p = _Plan()

    # --- assignment: group by r_e, round-robin to cores -------------------
    info = []
    for b in range(B):
        ts = int(bs[b]) - 1
        if ts < HS:
            r_e, bm = ts, 0
        else:
            bm = (ts // HS) * HS
            r_e = ts - bm + W
        info.append((r_e, ts, b, bm))
    info.sort()
    p.gidx = np.zeros((NCORES, BPC), np.int64)
    p.re = np.zeros((NCORES, BPC), np.int64)
    p.tstar = np.zeros((NCORES, BPC), np.int64)
    p.bm = np.zeros((NCORES, BPC), np.int64)
    for rank, (r_e, ts, b, bm) in enumerate(info):
        k, e = rank % NCORES, rank // NCORES
        p.gidx[k, e] = b
        p.re[k, e] = r_e
        p.tstar[k, e] = ts
        p.bm[k, e] = bm
    ext_death = p.re.max(axis=0)
    assert int(ext_death.max()) <= HS + W

    # --- per-element segment gap lists (8s and 16s), hop quota ------------
    QHOP = 4 * YP              # hop half-slots per core (2 chains x 2 halves)
    p.gaps = [[None] * BPC for _ in range(NCORES)]
    counts = {"B0": np.zeros(NCORES, np.int64),
              "A0": np.zeros(NCORES, np.int64),
              "D": np.zeros(NCORES, np.int64),
              "H": np.zeros(NCORES, np.int64)}
    for k in range(NCORES):
        q = QHOP
        # big elements first so quota parity always resolves
        order = sorted(range(BPC), key=lambda e: -int(p.bm[k, e]))
        for e in order:
            G = int(p.bm[k, e]) // HS
            # seg0's gap is free (B0/A0 column); only mid 8-gaps use hop
            # slots, so up to q+1 eights fit.  Parity: n8 must match G.
            n8 = min(G, q + 1)
            if (n8 - G) % 2:
                n8 -= 1
            if n8 < G % 2:
                n8 = G % 2
            q -= max(n8 - 1, 0)
            n16 = (G - n8) // 2
            # gap list in chain order: one 8 first if any (seg0 -> B0)
            if n8 >= 1:
                gaps = [8] * n8 + [16] * n16
            else:
                gaps = [16] * n16
            assert sum(gaps) == int(p.bm[k, e])
            p.gaps[k][e] = gaps
            if gaps:
                counts["B0" if gaps[0] == 8 else "A0"][k] += 1
                counts["H"][k] += (n8 - 1) if n8 >= 1 else 0
                counts["D"][k] += n16 if n8 >= 1 else n16 - 1
        assert q >= -1
    NB0 = int(max((int(n) + 1) // 2 for n in counts["B0"]))
    NA0 = int(max((int(n) + 1) // 2 for n in counts["A0"]))
    NDM = int(max((int(n) + 1) // 2 for n in counts["D"])) + 1

    # --- global death-sorted d-region columns -----------------------------
    cols = [(int(ext_death[e]), 0, e) for e in range(BPC)]
    cols += [(SNAPB, 1, i) for i in range(NB0)]
    cols.sort()
    cols += [(SNAPA, 2, i) for i in range(NA0)]
    cols += [(SL, 3, i) for i in range(NDM)]
    p.zs = len(cols)
    p.ncol = p.zs + 2 * YP
    p.col_death = np.array([cc[0] for cc in cols], np.int64)
    assert np.all(np.diff(p.col_death) >= 0)
    p.ext_col = np.zeros(BPC, np.int64)
    b0_cols, a0_cols = [], []
    dm0 = None
    for ci, (_, cls, ident) in enumerate(cols):
        if cls == 0:
            p.ext_col[ident] = ci
        elif cls == 1:
            b0_cols.append(ci)
        elif cls == 2:
            a0_cols.append(ci)
        elif dm0 is None:
            dm0 = ci
    p.b0_rng = (b0_cols[0], b0_cols[-1] + 1) if b0_cols else (0, 0)
    p.a0_rng = (a0_cols[0], a0_cols[-1] + 1) if a0_cols else (0, 0)
    p.dm0 = dm0 if dm0 is not None else p.zs

    p.d = np.array([int(np.searchsorted(p.col_death, r))
                    for r in range(L + 1)], np.int64)

    # --- per-core slot assignment ----------------------------------------
    # segs[k][e] = list of (t0, steps, kind, col, half); ext is separate
    p.segs = [[None] * BPC for _ in range(NCORES)]
    for k in range(NCORES):
        it_b0 = iter([(c, h) for c in b0_cols for h in (0, 1)])
        it_a0 = iter([(c, h) for c in a0_cols for h in (0, 1)])
        it_d = iter([(c, h) for c in range(p.dm0, p.zs) for h in (0, 1)])
        it_h = iter([(c, h) for c in range(p.zs, p.ncol) for h in (0, 1)])
        for e in range(BPC):
            gaps = p.gaps[k][e]
            segs = []
            b_cum = 0
            for j, g in enumerate(gaps):
                t0 = 0 if j == 0 else b_cum - W
                if j == 0:
                    kind = "B0" if g == 8 else "A0"
                    col, half = next(it_b0 if g == 8 else it_a0)
                    steps = SNAPB if g == 8 else SNAPA
                elif g == 16:
                    kind, (col, half), steps = "D", next(it_d), SL
                else:
                    kind, (col, half), steps = "H", next(it_h), HS
                segs.append((t0, steps, kind, col, half))
                b_cum += g
            p.segs[k][e] = segs

    # --- extraction copy ops ---------------------------------------------
    re_min = p.re.min(axis=0)
    re_max = p.re.max(axis=0)
    p.copies = []
    fin_off = 0
    for r in range(L):
        es = [e for e in range(BPC) if re_min[e] <= r <= re_max[e]]
        if not es:
            continue
        runs = []
        for e in es:
            cc = int(p.ext_col[e])
            if runs and e == runs[-1][1] + 1 and cc == runs[-1][3] + 1:
                runs[-1][1] = e
                runs[-1][3] = cc
            else:
                runs.append([e, e, cc, cc])
        for (e0, e1, c0, _c1) in runs:
            n = e1 - e0 + 1
            p.copies.append((r, e0, n, c0, fin_off))
            fin_off += n
    p.nfin = fin_off
    p.fincol = np.zeros((NCORES, BPC), np.int64)
    for k in range(NCORES):
        for e in range(BPC):
            r = int(p.re[k, e])
            for (rr, e0, n, c0, fa) in p.copies:
                if rr == r and e0 <= e < e0 + n:
                    p.fincol[k, e] = fa + (e - e0)
                    break
            else:
                raise AssertionError("no copy op for event")
    p.cum_copies = np.zeros(L + 1, np.int64)
    for r in range(L):
        p.cum_copies[r + 1] = p.cum_copies[r] + sum(
            1 for (rr, *_x) in p.copies if rr == r)

    # --- chain geometry ---------------------------------------------------
    p.mids = np.zeros(L, np.int64)
    for r in range(1, L):
        lo = int(p.d[r])
        m = (lo + p.zs) // 2
        assert m - lo <= 512 and p.zs - m <= 512
        p.mids[r] = m
    for (r, e0, n, c0, fa) in p.copies:
        if r >= 1:
            assert c0 + n <= int(p.mids[r])
    assert p.b0_rng[1] <= int(p.mids[SNAPB]) or p.b0_rng[1] == 0

    # --- X slab layout ----------------------------------------------------
    # round r slab: d-part [d(r), zs) + hop part (512 cols) for the active
    # chain: A steps on even rounds (2..16, step r/2), B on odd (1..15,
    # step (r+1)/2).  B's step 8 lands at round 15 so its end snapshot
    # ships during round 16.
    def hop_chain(r):
        if r >= 2 and r % 2 == 0:
            return 0
        if r >= 1 and r % 2 == 1:
            return 1
        return None
    p.hop_chain = hop_chain
    p.O = np.zeros(LB + 1, np.int64)
    p.O[1] = p.ncol                      # rawa
    for r in range(1, LB):
        wdt = p.zs - int(p.d[r])
        if hop_chain(r) is not None:
            wdt += YP
        p.O[r + 1] = p.O[r] + wdt
    p.ntot = int(p.O[LB])
    CB = [1, 2]
    while CB[-1] < LB:
        CB.append(min(CB[-1] + DCH, LB))
    p.CB = CB
    p.nchunk = len(CB) - 1
    p.chunk_of = [0] * LB
    for kk in range(p.nchunk):
        for r in range(CB[kk], CB[kk + 1]):
            p.chunk_of[r] = kk
    p.maxchunkw = max(int(p.O[CB[kk + 1]] - p.O[CB[kk]])
                      for kk in range(p.nchunk))

    # early/late fin split
    p.fin_split = 0
    p.fin_ops_early = 0
    for (r, e0, n, c0, fa) in p.copies:
        if r <= HS:
            p.fin_ops_early += 1
            p.fin_split = max(p.fin_split, fa + n)
    if p.fin_split > p.nfin - 4:
        p.fin_split = 0
    return p


def _build_host_inputs(p, pad_x, transitions, origination, c):
    import ml_dtypes
    mc = np.exp(np.asarray(transitions, np.float64) - c).astype(np.float32)
    wmat = np.zeros((128, 128), ml_dtypes.bfloat16)
    wmat[:64, :64] = mc.T.astype(ml_dtypes.bfloat16)
    wmat[64:, 64:] = mc.T.astype(ml_dtypes.bfloat16)

    x0 = np.asarray(pad_x, np.float32).copy()
    x0[:, 0, :] += np.asarray(origination, np.float32)[None, :]

    ncol, zs = p.ncol, p.zs
    xraw = np.empty((NCORES, 128, p.ntot), ml_dtypes.bfloat16)
    for k in range(NCORES):
        t0s = np.full((2, ncol), -10 ** 9, np.int64)
        bofs = np.zeros((2, ncol), np.int64)
        for e in range(BPC):
            gb = int(p.gidx[k, e])
            ts = int(p.tstar[k, e])
            t0s[0, int(p.ext_col[e])] = 0 if ts < HS else int(p.bm[k, e]) - W
            bofs[0, int(p.ext_col[e])] = gb
            for (t0, steps, kind, col, half) in p.segs[k][e]:
                t0s[half, col] = t0
                bofs[half, col] = gb
        # device steps per column: d-region cols see step r at round r;
        # hop cols see step s at round 2s (A) / 2s+1 (B).
        xfull = np.empty((128, L, ncol), np.float32)
        for hh in range(2):
            t_idx = t0s[hh][:, None] + np.arange(L)[None, :]
            valid = (t_idx >= 0) & (t_idx < T) & (t0s[hh][:, None] > -10**8)
            t_clip = np.clip(t_idx, 0, T - 1)
            blk = x0[bofs[hh][:, None], t_clip, :]
            blk = np.where(valid[:, :, None], blk, 0.0)
            xfull[64 * hh:64 * hh + 64] = np.exp(blk).transpose(2, 1, 0)
        out = xraw[k]
        out[:, 0:ncol] = xfull[:, 0, :].astype(ml_dtypes.bfloat16)
        for r in range(1, LB):
            o = int(p.O[r])
            d = int(p.d[r])
            out[:, o:o + zs - d] = \
                xfull[:, r, d:zs].astype(ml_dtypes.bfloat16)
            o += zs - d
            hc = p.hop_chain(r)
            if hc is not None:
                s = r // 2 if hc == 0 else (r + 1) // 2
                cb = zs + hc * YP
                out[:, o:o + YP] = \
                    xfull[:, s, cb:cb + YP].astype(ml_dtypes.bfloat16)
    return xraw, wmat


def _build_program(p):
    import concourse.bass as bass
    from concourse import mybir

    dt = mybir.dt
    ncol, zs = p.ncol, p.zs
    CB = p.CB

    nc = bass.Bass()
    xp = nc.declare_dram_parameter("xp", [128, p.ntot], dt.bfloat16, False)
    wm = nc.declare_dram_parameter("wm", [128, 128], dt.bfloat16, False)
    snapb = nc.declare_dram_parameter(
        "snapb", [128, max(p.b0_rng[1] - p.b0_rng[0], 1)], dt.bfloat16, True)
    snapa = nc.declare_dram_parameter(
        "snapa", [128, max(p.a0_rng[1] - p.a0_rng[0], 1)], dt.bfloat16, True)
    snapf = nc.declare_dram_parameter(
        "snapf", [128, zs + YP - p.dm0], dt.bfloat16, True)
    snapfb = nc.declare_dram_parameter(
        "snapfb", [128, YP], dt.bfloat16, True)
    fin = nc.declare_dram_parameter("fin", [64, p.nfin], dt.bfloat16, True)

    with ExitStack() as ctx:
        def sb(name, shape, d):
            return ctx.enter_context(nc.sbuf_tensor(name, shape, d))
        w = sb("w", [128, 128], dt.bfloat16)
        rawa = sb("rawa", [128, ncol], dt.bfloat16)
        raw = [sb(f"raw{i}", [128, p.maxchunkw], dt.bfloat16)
               for i in range(RING)]
        et = [sb(f"et{i}", [128, ncol], dt.bfloat16) for i in range(ED)]
        hbp = [sb(f"hbp{i}", [128, YP], dt.bfloat16) for i in range(2)]
        fin_t = sb("fin_t", [64, p.nfin], dt.bfloat16)
        psd = [ctx.enter_context(
            nc.psum_tensor(f"psd{cidx}", [128, 512], dt.float32))
            for cidx in range(2)]
        psp = [ctx.enter_context(
            nc.psum_tensor(f"psp{i}", [128, 512], dt.float32))
            for i in range(2)]
        psf = ctx.enter_context(nc.psum_tensor("psf", [128, 128], dt.float32))
        s_w = ctx.enter_context(nc.semaphore("s_w"))
        s_xa = ctx.enter_context(nc.semaphore("s_xa"))
        s_xh = ctx.enter_context(nc.semaphore("s_xh"))
        s_x = tuple(ctx.enter_context(nc.semaphore(f"s_x{i}"))
                    for i in range(NSX))
        s_pd = ctx.enter_context(nc.semaphore("s_pd"))
        s_pp = ctx.enter_context(nc.semaphore("s_pp"))
        s_hp = ctx.enter_context(nc.semaphore("s_hp"))
        s_vd = ctx.enter_context(nc.semaphore("s_vd"))
        s_vp = ctx.enter_context(nc.semaphore("s_vp"))
        s_f = ctx.enter_context(nc.semaphore("s_f"))
        s_o = ctx.enter_context(nc.semaphore("s_o"))
        block = ctx.enter_context(nc.Block())

        def drng(r):
            lo, m = int(p.d[r]), int(p.mids[r])
            return ((lo, m - lo), (m, zs - m))

        def xsl_d(r, c0, n):
            kk = p.chunk_of[r]
            off = int(p.O[r] - p.O[CB[kk]]) + (c0 - int(p.d[r]))
            return raw[kk % RING][:, off:off + n]

        def xsl_h(r, hc):
            kk = p.chunk_of[r]
            off = int(p.O[r] - p.O[CB[kk]]) + \
                ((zs - int(p.d[r])) if r <= SL else 0)
            return raw[kk % RING][:, off:off + YP]

        def chunk_arrived(eng, r):
            kk = p.chunk_of[r]
            eng.wait_ge(s_x[kk % NSX], 16 * (kk // NSX + 1))

        @block.sync
        def _(sync):
            sync.dma_start(w[:], wm[:, :]).then_inc(s_w, 16)
            sync.dma_start(rawa[:, 0:zs], xp[:, 0:zs]).then_inc(s_xa, 16)
            # chunk 0's d-part jumps the queue so round 1 starts early
            o1, o2 = int(p.O[1]), int(p.O[2])
            wd1 = zs - int(p.d[1])
            sync.dma_start(raw[0][:, :wd1],
                           xp[:, o1:o1 + wd1]).then_inc(s_x[0], 16)
            sync.dma_start(rawa[:, zs:ncol],
                           xp[:, zs:ncol]).then_inc(s_xa, 16)
            sync.dma_start(raw[0][:, wd1:o2 - o1],
                           xp[:, o1 + wd1:o2]).then_inc(s_xh, 16)
            for kk in range(1, p.nchunk):
                if kk >= RING:
                    r_last = CB[kk - RING + 1] - 1
                    sync.wait_ge(s_vd, 2 * min(r_last, SL))
                    sync.wait_ge(s_vp, r_last)
                if kk >= NSX:
                    sync.wait_ge(s_x[kk % NSX], 16 * (kk // NSX))
                o0, o1 = int(p.O[CB[kk]]), int(p.O[CB[kk + 1]])
                sync.dma_start(
                    raw[kk % RING][:, :o1 - o0], xp[:, o0:o1],
                ).then_inc(s_x[kk % NSX], 16)
            if p.b0_rng[1] > p.b0_rng[0]:
                sync.wait_ge(s_vd, 2 * SNAPB)
                sync.dma_start(
                    snapb[:],
                    et[SNAPB % ED][:, p.b0_rng[0]:p.b0_rng[1]],
                ).then_inc(s_o, 16)
            if p.a0_rng[1] > p.a0_rng[0]:
                sync.wait_ge(s_vd, 2 * SNAPA)
                sync.dma_start(
                    snapa[:],
                    et[SNAPA % ED][:, p.a0_rng[0]:p.a0_rng[1]],
                ).then_inc(s_o, 16)
            if p.fin_split:
                sync.wait_ge(s_f, p.fin_ops_early)
                sync.dma_start(fin[:, 0:p.fin_split],
                               fin_t[:, 0:p.fin_split]).then_inc(s_o, 16)
                sync.wait_ge(s_f, len(p.copies))
                sync.dma_start(fin[:, p.fin_split:],
                               fin_t[:, p.fin_split:]).then_inc(s_o, 16)
            else:
                sync.wait_ge(s_f, len(p.copies))
                sync.dma_start(fin[:, :], fin_t[:]).then_inc(s_o, 16)
            # hop-B end snapshot (step 8 at round 15) ships during round 16
            sync.wait_ge(s_vp, SL - 1)
            sync.dma_start(
                snapfb[:], et[(SL - 1) % ED][:, zs + YP:ncol]).then_inc(s_o, 16)
            # d-segment end snapshot right after the last d-muls, then the
            # hop-A part once pool finishes round 16 (pipelines the DMA
            # fixed costs with the last pool mul)
            sync.wait_ge(s_vd, 2 * SL)
            sync.dma_start(
                snapf[:, 0:zs - p.dm0],
                et[SL % ED][:, p.dm0:zs]).then_inc(s_o, 16)
            sync.wait_ge(s_vp, SL)
            sync.dma_start(
                snapf[:, zs - p.dm0:],
                et[SL % ED][:, zs:zs + YP]).then_inc(s_o, 16)

        @block.scalar
        def _(scalar):
            copies_by_round = {}
            for (r, e0, n, c0, fa) in p.copies:
                copies_by_round.setdefault(r, []).append((e0, n, c0, fa))
            if 0 in copies_by_round:
                scalar.wait_ge(s_xa, 16)
                for (e0, n, c0, fa) in copies_by_round[0]:
                    nc.scalar.copy(fin_t[:, fa:fa + n],
                                   rawa[0:64, c0:c0 + n]).then_inc(s_f, 1)
            for r in range(1, LB):
                hc = p.hop_chain(r)
                if hc is not None:
                    scalar.wait_ge(s_pp, r)
                    nc.scalar.copy(hbp[hc][:],
                                   psp[hc][:, :YP]).then_inc(s_hp, 1)
                if r in copies_by_round:
                    scalar.wait_ge(s_vd, 2 * (r - 1) + 1)
                    for (e0, n, c0, fa) in copies_by_round[r]:
                        nc.scalar.copy(
                            fin_t[:, fa:fa + n],
                            et[r % ED][0:64, c0:c0 + n]).then_inc(s_f, 1)

        @block.tensor
        def _(tensor):
            def filler(n=1):
                for _ in range(n):
                    nc.tensor.matmul(psf[:], w[:], w[:, 0:128],
                                     start=True, stop=True)

            tensor.wait_ge(s_w, 16)
            filler(PRE_FILL)
            for r in range(1, LB):
                for cidx, (c0, n) in enumerate(drng(r)):
                    if r == 1:
                        if cidx == 0:
                            tensor.wait_ge(s_xa, 16)
                        mov = rawa[:, c0:c0 + n]
                    else:
                        tensor.wait_ge(s_vd, 2 * (r - 1) + cidx - 1)
                        mov = et[(r - 1) % ED][:, c0:c0 + n]
                    nc.tensor.matmul(
                        psd[cidx][:, :n], w[:], mov,
                        start=True, stop=True).then_inc(s_pd, 1)
                hc = p.hop_chain(r)
                if hc is not None:
                    cb = zs + hc * YP
                    if r <= 2:
                        if r == 1:
                            tensor.wait_ge(s_xa, 32)
                        mov = rawa[:, cb:cb + YP]
                    else:
                        tensor.wait_ge(s_vp, r - 2)
                        mov = et[(r - 2) % ED][:, cb:cb + YP]
                    nc.tensor.matmul(psp[hc][:, :YP], w[:], mov,
                                     start=True, stop=True).then_inc(s_pp, 1)
                filler(FILLERS)

        @block.vector
        def _(vector):
            for r in range(1, L):
                if r == CB[p.chunk_of[r]]:
                    chunk_arrived(vector, r)
                for cidx, (c0, n) in enumerate(drng(r)):
                    if cidx == 0:
                        if r >= ED and p.cum_copies[r - ED + 1] > \
                                p.cum_copies[r - ED]:
                            vector.wait_ge(s_f, int(p.cum_copies[r - ED + 1]))
                        if r - ED == SNAPB and p.b0_rng[1] > p.b0_rng[0]:
                            vector.wait_ge(s_o, 16)
                    vector.wait_ge(s_pd, 2 * (r - 1) + cidx + 1)
                    nc.vector.tensor_mul(
                        et[r % ED][:, c0:c0 + n],
                        psd[cidx][:, :n],
                        xsl_d(r, c0, n)).then_inc(s_vd, 1)

        @block.gpsimd
        def _(gpsimd):
            for r in range(1, LB):
                hc = p.hop_chain(r)
                if hc is None:
                    continue
                if r == 1:
                    gpsimd.wait_ge(s_xh, 16)
                elif p.chunk_of[r] != p.chunk_of[r - 1]:
                    chunk_arrived(gpsimd, r)
                cb = zs + hc * YP
                gpsimd.wait_ge(s_hp, r)
                nc.gpsimd.scalar_tensor_tensor(
                    et[r % ED][:, cb:cb + YP],
                    hbp[hc][:],
                    1.0,
                    xsl_h(r, hc),
                    mybir.AluOpType.mult,
                    mybir.AluOpType.mult).then_inc(s_vp, 1)

    return nc


def _postprocess(p, k, outs, xraw_k, c):
    """Host math for core k: stitch offsets, read finals (float64)."""
    lx0 = np.log(np.maximum(
        np.asarray(xraw_k[:, 0:p.ncol], np.float64), 1e-300))
    lsb = np.log(np.maximum(np.asarray(outs["snapb"], np.float64), 1e-300))
    lsa = np.log(np.maximum(np.asarray(outs["snapa"], np.float64), 1e-300))
    lsf = np.log(np.maximum(np.asarray(outs["snapf"], np.float64), 1e-300))
    lsfb = np.log(np.maximum(np.asarray(outs["snapfb"], np.float64), 1e-300))
    lf = np.log(np.maximum(np.asarray(outs["fin"], np.float64), 1e-300))

    def vec(arr, rng0, col, half):
        return arr[64 * half:64 * half + 64, col - rng0]

    res = np.empty(BPC)
    for e in range(BPC):
        segs = p.segs[k][e]
        r_e = int(p.re[k, e])
        A = 0.0
        for j in range(len(segs)):
            t0, steps, kind, col, half = segs[j]
            if kind == "B0":
                prev = vec(lsb, p.b0_rng[0], col, half) + SNAPB * c
            elif kind == "A0":
                prev = vec(lsa, p.a0_rng[0], col, half) + SNAPA * c
            elif kind == "D":
                prev = vec(lsf, p.dm0, col, half) + SL * c
            elif col < p.zs + YP:
                prev = vec(lsf, p.dm0, col, half) + HS * c
            else:
                prev = vec(lsfb, p.zs + YP, col, half) + HS * c
            # cur = raw init of the NEXT segment (or the extraction segment)
            if j + 1 < len(segs):
                ncol_, nhalf = segs[j + 1][3], segs[j + 1][4]
            else:
                ncol_, nhalf = int(p.ext_col[e]), 0
            cur = vec(lx0, 0, ncol_, nhalf)
            A += (prev - cur).mean()
        res[e] = lf[:, int(p.fincol[k, e])].sum() + 64.0 * (r_e * c + A)
    return res


def kernel(pad_x, transitions, origination, batch_sizes):
    from concourse.bass_utils import run_bass_kernel_spmd

    pad_x = np.asarray(pad_x)
    transitions = np.asarray(transitions)
    origination = np.asarray(origination)
    batch_sizes = np.asarray(batch_sizes)

    c = _c_step(transitions, pad_x)
    p = _plan(batch_sizes)
    xraw, wmat = _build_host_inputs(p, pad_x, transitions, origination, c)

    key = batch_sizes.tobytes()
    if key not in _CACHE:
        _CACHE[key] = _build_program(p)
    nc = _CACHE[key]

    in_maps = [{"xp": xraw[i], "wm": wmat} for i in range(NCORES)]
    out = run_bass_kernel_spmd(nc, in_maps, list(range(NCORES)))

    res = np.empty(B, np.float32)
    for k in range(NCORES):
        r = _postprocess(p, k, out.results[k], xraw[k], c)
        for e in range(BPC):
            res[int(p.gidx[k, e])] = np.float32(r[e])
    return res
